# revision 1
# baseline (speedup 1.0000x reference)
"""Trainium2 Bass kernel for CellGraphSignatureGNN (GCN message passing).

Math: the network is affine per layer: x_{l+1} = A @ x_l @ W_l + 1 b_l^T,
with A = D^-1/2 (Adj + 2I) D^-1/2 (weighted adjacency + improved self loops),
followed by a per-graph mean pool P (and bs == 0 in this problem).  Since A
acts on nodes and W on features:

    out = P A^3 X (W0 W1 W2) / counts  (+ rank-1 bias terms)

We evaluate left-to-right: u1^T = A^T P^T, u2^T = A^T u1^T, u3^T = A^T u2^T
(64-wide node vectors), then one dense matmul (u3 X) sharded over nodes, a
tiny AllReduce, and the 128x128 weight chain on-device.

Distribution: nodes are re-labeled and bin-packed into 128-node blocks (98
blocks/core x 8 cores) balanced by scatter-degree so every core runs an
IDENTICAL (SPMD) program; per-core data (gather indices, one-hot offsets,
norms) differ.  Edge (r, c) contributes norm_e * u[c] into u_new[r]:
  - gather u[c] rows (256B fp32) from the replicated u in HBM via dma_gather
    (int16 indices => 4 source windows of 25088 rows),
  - scale by norm_e on DVE (bf16 out), build a destination one-hot on DVE,
  - scatter = one-hot matmul accumulated in PSUM per 128-dest block,
  - per layer AllGather of the 12544x64 fp32 shard re-replicates u.
Self-loops are applied as local elementwise ops; rare overflow edges go
through a "virtual block" + indirect scatter-add with CCE.
"""

import numpy as np
import ml_dtypes

BF16 = ml_dtypes.bfloat16

G = 64        # graphs
F = 128       # feature width
LAYERS = 3
PAD_SENT = 30000.0  # destoff/bg sentinel for padded slots (one-hot -> 0)


# --------------------------------------------------------------------------
# configuration
# --------------------------------------------------------------------------
class Cfg:
    def __init__(self, n_nodes, n_edges, n_cores=8, nblk=98, group_sizes=None,
                 seg_chunks=2, n_win=4, vchunks=2, vcap=128):
        self.n_nodes = n_nodes
        self.n_edges = n_edges
        self.n_cores = n_cores
        self.nblk = nblk                      # real blocks per core
        self.group_sizes = group_sizes or [14] * 7
        assert sum(self.group_sizes) == nblk
        self.seg_chunks = seg_chunks          # 128-slot chunks per (block, window)
        self.n_win = n_win
        self.vchunks = vchunks                # chunks per window for the virtual blk
        self.vcap = vcap                      # distinct spill destinations
        self.core_rows = nblk * 128           # permuted rows per core
        self.pn = n_cores * self.core_rows    # total permuted rows
        assert self.pn % n_win == 0
        self.win = self.pn // n_win
        assert self.win <= 32768
        # slot layout: for g in groups + [vgroup]: for w in windows: contig slots
        self.groups = []
        off = 0
        for gs in self.group_sizes:
            self.groups.append(gs)
        # chunks per (group, window)
        self.gw_chunks = [gs * seg_chunks for gs in self.group_sizes] + [vchunks]
        self.n_groups = len(self.gw_chunks)   # includes virtual group
        self.slots_total = sum(c for c in self.gw_chunks) * 128 * n_win
        # slot offset of (g, w)
        self.gw_slot_off = {}
        off = 0
        for g, c in enumerate(self.gw_chunks):
            for w in range(n_win):
                self.gw_slot_off[(g, w)] = off
                off += c * 128
        assert off == self.slots_total


FULL_CFG = Cfg(100000, 640000)


# --------------------------------------------------------------------------
# host-side graph preprocessing (indices, norms, schedules)
# --------------------------------------------------------------------------
def host_prep(cfg, x, edge_index, edge_attr, batch, Ws, bs):
    N, E = cfg.n_nodes, cfg.n_edges
    row = np.asarray(edge_index[0], dtype=np.int64)
    col = np.asarray(edge_index[1], dtype=np.int64)
    w = np.asarray(edge_attr, dtype=np.float32).reshape(-1)
    batch = np.asarray(batch, dtype=np.int64)

    deg = np.zeros(N, dtype=np.float64)
    np.add.at(deg, col, w.astype(np.float64))
    deg += 2.0
    dinv = (1.0 / np.sqrt(deg)).astype(np.float32)
    norm = dinv[row] * w * dinv[col]
    selfnorm = 2.0 * dinv * dinv
    cnt = np.bincount(batch, minlength=G).astype(np.float32)

    # ---- bin-pack nodes into blocks by scatter degree (edges with row == n)
    sdeg = np.bincount(row, minlength=N)
    nbins = cfg.n_cores * cfg.nblk
    order = np.argsort(-sdeg, kind="stable")
    binsum = np.zeros(nbins, dtype=np.int64)
    binfill = np.zeros(nbins, dtype=np.int32)
    import heapq
    heap = [(0, b) for b in range(nbins)]
    heapq.heapify(heap)
    node_bin = np.empty(N, dtype=np.int32)
    node_pos = np.empty(N, dtype=np.int32)
    for n in order:
        while True:
            s, b = heapq.heappop(heap)
            if binfill[b] < 128:
                break
        node_bin[n] = b
        node_pos[n] = binfill[b]
        binfill[b] += 1
        binsum[b] += sdeg[n]
        if binfill[b] < 128:
            heapq.heappush(heap, (int(binsum[b]), b))
    # snake-assign bins to cores by load
    border = np.argsort(-binsum, kind="stable")
    bin_core = np.empty(nbins, dtype=np.int32)
    bin_blk = np.empty(nbins, dtype=np.int32)
    percore = [[] for _ in range(cfg.n_cores)]
    for i, b in enumerate(border):
        r = i // cfg.n_cores
        k = i % cfg.n_cores
        c = k if (r % 2 == 0) else cfg.n_cores - 1 - k
        bin_core[b] = c
        bin_blk[b] = len(percore[c])
        percore[c].append(b)
    assert all(len(p) == cfg.nblk for p in percore)

    perm = (bin_core[node_bin].astype(np.int64) * cfg.core_rows
            + bin_blk[node_bin].astype(np.int64) * 128 + node_pos)

    # ---- per-core edge schedules
    e_core = bin_core[node_bin[row]]
    e_blk = bin_blk[node_bin[row]]
    e_doff = node_pos[row]                 # dest offset within block
    e_win = (perm[col] // cfg.win).astype(np.int32)
    e_gidx = (perm[col] % cfg.win).astype(np.int32)

    SEG = cfg.seg_chunks * 128
    S = cfg.slots_total
    n_cores = cfg.n_cores

    gidx = np.zeros((n_cores, S), dtype=np.int32)
    doff = np.full((n_cores, S), PAD_SENT, dtype=np.float32)
    nrm = np.zeros((n_cores, S), dtype=np.float32)
    bg = np.full((n_cores, S), PAD_SENT, dtype=np.float32)

    # block -> (group, block-in-group) mapping
    blk_group = []
    blk_ing = []
    for g, gs in enumerate(cfg.group_sizes):
        for j in range(gs):
            blk_group.append(g)
            blk_ing.append(j)
    blk_group = np.array(blk_group)
    blk_ing = np.array(blk_ing)

    vmaps = np.full((n_cores, 128, 1), 1 << 30, dtype=np.int32)
    spill_warn = 0

    for c in range(n_cores):
        em = e_core == c
        eb = e_blk[em]
        ew = e_win[em]
        eg = e_gidx[em]
        ed = e_doff[em]
        en = norm[em]
        ebg = batch[col[em]].astype(np.float32)
        # order edges by (block, window)
        key = eb * cfg.n_win + ew
        o = np.argsort(key, kind="stable")
        eb, ew, eg, ed, en, ebg = eb[o], ew[o], eg[o], ed[o], en[o], ebg[o]
        # segment boundaries
        spill_list = []
        kk = eb * cfg.n_win + ew
        bounds = np.searchsorted(kk, np.arange(cfg.nblk * cfg.n_win + 1))
        vused = {}
        for b in range(cfg.nblk):
            g = blk_group[b]
            j = blk_ing[b]
            for wi in range(cfg.n_win):
                lo, hi = bounds[b * cfg.n_win + wi], bounds[b * cfg.n_win + wi + 1]
                nseg = hi - lo
                take = min(nseg, SEG)
                base = (cfg.gw_slot_off[(g, wi)] + j * SEG)
                sl = slice(base, base + take)
                gidx[c, sl] = eg[lo:lo + take]
                doff[c, sl] = ed[lo:lo + take]
                nrm[c, sl] = en[lo:lo + take]
                bg[c, sl] = ebg[lo:lo + take]
                for t in range(lo + take, hi):
                    spill_list.append((b, ew[t], eg[t], ed[t], en[t], ebg[t]))
        # spills into virtual group
        vg = cfg.n_groups - 1
        vfill = np.zeros(cfg.n_win, dtype=np.int32)
        for (b, wi, gg, dd, nn, bb) in spill_list:
            key2 = (b, dd)
            if key2 not in vused:
                assert len(vused) < cfg.vcap, "virtual dest capacity exceeded"
                v = len(vused)
                vused[key2] = v
                vmaps[c, v, 0] = b * 128 + dd
            v = vused[key2]
            assert vfill[wi] < cfg.vchunks * 128, "virtual slot capacity exceeded"
            base = cfg.gw_slot_off[(vg, wi)] + vfill[wi]
            gidx[c, base] = gg
            doff[c, base] = v
            nrm[c, base] = nn
            bg[c, base] = bb
            vfill[wi] += 1
        spill_warn += len(spill_list)

    # ---- pack aux arrays
    ncol16 = S // 16
    gidx16 = np.zeros((n_cores, 128, ncol16), dtype=np.int16)
    s_idx = np.arange(S)
    for c in range(n_cores):
        lay = np.zeros((16, ncol16), dtype=np.int16)
        lay[s_idx % 16, s_idx // 16] = gidx[c].astype(np.int16)
        gidx16[c] = np.tile(lay, (8, 1))
    ncol128 = S // 128
    def slotmajor(a, dt):
        out = np.zeros((n_cores, 128, ncol128), dtype=dt)
        for c in range(n_cores):
            out[c][s_idx % 128, s_idx // 128] = a[c]
        return out
    nrm_sm = slotmajor(nrm, np.float32)
    doff_sm = slotmajor(doff, BF16)
    bg_sm = slotmajor(bg, BF16)

    # ---- per-core node-level aux
    selfw = np.zeros((n_cores, 128, cfg.nblk), dtype=np.float32)
    batchloc = np.full((n_cores, 128, cfg.nblk), PAD_SENT, dtype=BF16)
    Xp = np.zeros((n_cores, cfg.core_rows, F), dtype=np.float32)
    nodes = np.arange(N)
    pc = bin_core[node_bin]
    pb = bin_blk[node_bin]
    pp = node_pos
    for c in range(n_cores):
        m = pc == c
        selfw[c][pp[m], pb[m]] = selfnorm[nodes[m]]
        batchloc[c][pp[m], pb[m]] = batch[nodes[m]].astype(np.float32)
        Xp[c][pb[m] * 128 + pp[m]] = np.asarray(x, dtype=np.float32)[nodes[m]]

    inv_cnt = (1.0 / np.maximum(cnt, 1.0)).astype(np.float32).reshape(G, 1)
    cnt_row = cnt.reshape(1, G)
    Ws = np.asarray(Ws, dtype=np.float32)
    bs = np.asarray(bs, dtype=np.float32)

    aux = dict(
        gidx16=gidx16, nrm_sm=nrm_sm, doff_sm=doff_sm, bg_sm=bg_sm,
        selfw=selfw, batchloc=batchloc, Xp=Xp, vmaps=vmaps,
        inv_cnt=inv_cnt, cnt_row=cnt_row,
        W0T=np.ascontiguousarray(Ws[0].T), W1T=np.ascontiguousarray(Ws[1].T),
        W2=np.ascontiguousarray(Ws[2]), bs=bs,
        perm=perm, spills=spill_warn,
    )
    return aux


# --------------------------------------------------------------------------
# numpy emulation of the device program (for validation)
# --------------------------------------------------------------------------
def _midbcast(ap, count):
    """Insert a step-0 middle axis: [P, X] -> [P, (0,count), X]."""
    import concourse.bass as bass
    assert len(ap.ap) == 2
    return bass.AP(ap.tensor, ap.offset, [ap.ap[0], [0, count], ap.ap[1]])


def build_program(cfg, bias_nonzero=False, dbg=False):
    import contextlib
    import concourse.bacc as bacc
    import concourse.bass as bass
    import concourse.mybir as mybir
    import concourse.tile as tile

    f32 = mybir.dt.float32
    bf16 = mybir.dt.bfloat16
    i16 = mybir.dt.int16
    i32 = mybir.dt.int32
    AL = mybir.AluOpType

    S = cfg.slots_total
    CR = cfg.core_rows
    NBLK = cfg.nblk
    NW = cfg.n_win
    SEGC = cfg.seg_chunks
    gbase = [0]
    for gs in cfg.group_sizes:
        gbase.append(gbase[-1] + gs)

    nc = bacc.Bacc("TRN2", debug=False, num_devices=cfg.n_cores)
    P = nc.declare_dram_parameter

    gidx16 = P("gidx16", [128, S // 16], i16, isOutput=False)
    nrm_sm = P("nrm_sm", [128, S // 128], f32, isOutput=False)
    doff_sm = P("doff_sm", [128, S // 128], bf16, isOutput=False)
    bg_sm = P("bg_sm", [128, S // 128], bf16, isOutput=False)
    selfw = P("selfw", [128, NBLK], f32, isOutput=False)
    batchloc = P("batchloc", [128, NBLK], bf16, isOutput=False)
    Xp = P("Xp", [CR, F], f32, isOutput=False)
    vmaps = P("vmaps", [128, 1], i32, isOutput=False)
    inv_cnt = P("inv_cnt", [G, 1], f32, isOutput=False)
    W0T = P("W0T", [F, F], f32, isOutput=False)
    W1T = P("W1T", [F, F], f32, isOutput=False)
    W2 = P("W2", [F, F], f32, isOutput=False)
    out_ext = P("out", [G, F], f32, isOutput=True)
    dbg_sh = [P(f"dbg_sh{i}", [cfg.core_rows, G], f32, isOutput=True)
              for i in range(LAYERS)] if dbg else None

    shard = nc.dram_tensor("shard", [CR, G], f32)
    ufull = [nc.dram_tensor(f"ufull{i}", [cfg.pn, G], f32) for i in range(2)]
    arin = nc.dram_tensor("arin", [G, F], f32)
    arout = nc.dram_tensor("arout", [G, F], f32)

    iota128_c = nc.inline_tensor(
        np.tile(np.arange(128, dtype=np.float32).astype(BF16), (128, 1)), "iota128")
    iota64_c = nc.inline_tensor(
        np.tile(np.arange(G, dtype=np.float32).astype(BF16), (128, 1)), "iota64")
    ident_c = nc.inline_tensor(np.eye(128, dtype=np.float32), "ident")

    with tile.TileContext(nc) as tc:
        with contextlib.ExitStack() as ctx:
            perm_pool = ctx.enter_context(tc.tile_pool(name="perm", bufs=1))
            acc = perm_pool.tile([128, NBLK, G], f32, tag="acc")
            uprev = perm_pool.tile([128, NBLK, G], f32, tag="uprev")
            selfw_sb = perm_pool.tile([128, NBLK], f32, tag="selfw")
            io128 = perm_pool.tile([128, 128], bf16, tag="io128")
            io64 = perm_pool.tile([128, G], bf16, tag="io64")
            ident_sb = perm_pool.tile([128, 128], f32, tag="ident")
            vmap_sb = perm_pool.tile([128, 1], i32, tag="vmap")
            w_sb = perm_pool.tile([128, 3 * F], f32, tag="wsb")

            nc.sync.dma_start(out=selfw_sb[:], in_=selfw[:])
            nc.sync.dma_start(out=io128[:], in_=iota128_c[:])
            nc.sync.dma_start(out=io64[:], in_=iota64_c[:])
            nc.sync.dma_start(out=ident_sb[:], in_=ident_c[:])
            nc.sync.dma_start(out=vmap_sb[:], in_=vmaps[:])
            nc.sync.dma_start(out=w_sb[:, 0:F], in_=W0T[:])
            nc.sync.dma_start(out=w_sb[:, F:2 * F], in_=W1T[:])
            nc.sync.dma_start(out=w_sb[:, 2 * F:3 * F], in_=W2[:])

            idx_pool = ctx.enter_context(tc.tile_pool(name="idx", bufs=3))
            aux_pool = ctx.enter_context(tc.tile_pool(name="aux", bufs=3))
            raw_pool = ctx.enter_context(tc.tile_pool(name="raw", bufs=3))
            msg_pool = ctx.enter_context(tc.tile_pool(name="msg", bufs=3))
            oh_pool = ctx.enter_context(tc.tile_pool(name="oh", bufs=3))
            ps_pool = ctx.enter_context(tc.tile_pool(name="ps", bufs=2, space="PSUM"))
            ep_pool = ctx.enter_context(tc.tile_pool(name="ep", bufs=2, space="PSUM"))
            fin_pool = ctx.enter_context(tc.tile_pool(name="fin", bufs=2))
            xp_pool = ctx.enter_context(tc.tile_pool(name="xp", bufs=3))

            shard_pbf = shard[:].rearrange("(b p) f -> p b f", p=128)

            def do_pass(pk):
                src = ufull[(pk + 1) % 2]  # written by the previous pass's AG
                if pk == 0:
                    blc = aux_pool.tile([128, NBLK], bf16, tag="blc")
                    nc.sync.dma_start(out=blc[:], in_=batchloc[:])
                    nc.vector.tensor_tensor(
                        out=uprev[:, :, :],
                        in0=blc[:].to_broadcast([128, NBLK, G]),
                        in1=_midbcast(io64[:], NBLK),
                        op=AL.is_equal)
                else:
                    nc.sync.dma_start(out=uprev[:, :, :], in_=shard_pbf)

                vsb = None
                for g in range(cfg.n_groups):
                    is_virt = g == cfg.n_groups - 1
                    gs = 1 if is_virt else cfg.group_sizes[g]
                    C = cfg.gw_chunks[g]
                    if not is_virt:
                        # init acc segment with the self-loop term, then add
                        # each window's PSUM partial on DVE
                        seg = slice(gbase[g], gbase[g] + gs)
                        nc.vector.tensor_tensor(
                            out=acc[:, seg, :], in0=uprev[:, seg, :],
                            in1=selfw_sb[:, seg].to_broadcast([128, gs, G]),
                            op=AL.mult)
                    for w in range(NW):
                        ps = ps_pool.tile([128, max(cfg.group_sizes) * G], f32,
                                          tag="ps")
                        soff = cfg.gw_slot_off[(g, w)]
                        nslots = C * 128
                        coff = soff // 128
                        msgt = msg_pool.tile([128, C, G], bf16, tag="msg")
                        nrmt = aux_pool.tile([128, C], f32, tag="nrm")
                        nc.sync.dma_start(out=nrmt[:], in_=nrm_sm[:, coff:coff + C])
                        dofft = aux_pool.tile([128, C], bf16, tag="doff")
                        nc.sync.dma_start(out=dofft[:], in_=doff_sm[:, coff:coff + C])
                        if pk == 0:
                            bgt = aux_pool.tile([128, C], bf16, tag="bg")
                            nc.sync.dma_start(out=bgt[:], in_=bg_sm[:, coff:coff + C])
                            eqt = raw_pool.tile([128, C, G], f32, tag="raw")
                            nc.vector.tensor_tensor(
                                out=eqt[:, :, :],
                                in0=bgt[:].to_broadcast([128, C, G]),
                                in1=_midbcast(io64[:], C),
                                op=AL.is_equal)
                            nc.vector.tensor_tensor(
                                out=msgt[:, :, :], in0=eqt[:, :, :],
                                in1=nrmt[:].to_broadcast([128, C, G]),
                                op=AL.mult)
                        else:
                            idxt = idx_pool.tile([128, C * 8], i16, tag="idx")
                            nc.sync.dma_start(
                                out=idxt[:],
                                in_=gidx16[:, soff // 16:soff // 16 + C * 8])
                            rawt = raw_pool.tile([128, C, G], f32, tag="raw")
                            CSUB = 8  # chunks per dma_gather call (1024 idxs)
                            for sub in range(0, C, CSUB):
                                cs = min(CSUB, C - sub)
                                nc.gpsimd.dma_gather(
                                    rawt[:, sub:sub + cs, :],
                                    src[w * cfg.win:(w + 1) * cfg.win, :],
                                    idxt[:, sub * 8:(sub + cs) * 8],
                                    cs * 128, cs * 128, G,
                                    single_packet=False)
                            nc.vector.tensor_tensor(
                                out=msgt[:, :, :], in0=rawt[:, :, :],
                                in1=nrmt[:].to_broadcast([128, C, G]),
                                op=AL.mult)
                        oht = oh_pool.tile([128, C, 128], bf16, tag="oh")
                        nc.vector.tensor_tensor(
                            out=oht[:, :, :],
                            in0=dofft[:].to_broadcast([128, C, 128]),
                            in1=_midbcast(io128[:], C),
                            op=AL.is_equal)
                        for ci in range(C):
                            j = 0 if is_virt else ci // SEGC
                            first = ci == 0 if is_virt else ci % SEGC == 0
                            last = (ci == C - 1 if is_virt
                                    else ci % SEGC == SEGC - 1)
                            nc.tensor.matmul(
                                ps[:, j * G:(j + 1) * G],
                                lhsT=oht[:, ci, :], rhs=msgt[:, ci, :],
                                start=first, stop=last)
                        if is_virt:
                            if w == 0:
                                vsb = fin_pool.tile([128, G], f32, tag="vsb")
                                nc.vector.tensor_copy(out=vsb[:], in_=ps[:, :G])
                            else:
                                nc.vector.tensor_tensor(
                                    out=vsb[:], in0=vsb[:], in1=ps[:, :G],
                                    op=AL.add)
                        else:
                            nc.vector.tensor_tensor(
                                out=acc[:, seg, :], in0=acc[:, seg, :],
                                in1=ps[:, :gs * G].rearrange(
                                    "p (b f) -> p b f", f=G),
                                op=AL.add)
                return vsb

            for pk in range(LAYERS):
                vsb = do_pass(pk)
                nc.sync.dma_start(out=shard_pbf, in_=acc[:, :, :])
                nc.gpsimd.indirect_dma_start(
                    out=shard[:, :],
                    out_offset=bass.IndirectOffsetOnAxis(ap=vmap_sb[:, :1], axis=0),
                    in_=vsb[:, :], in_offset=None,
                    bounds_check=CR - 1, oob_is_err=False,
                    compute_op=AL.add)
                if dbg:
                    dtile = fin_pool.tile([128, NBLK, G], f32, tag="dbg")
                    nc.sync.dma_start(out=dtile[:, :, :], in_=shard_pbf)
                    nc.sync.dma_start(
                        out=dbg_sh[pk][:].rearrange("(b p) f -> p b f", p=128),
                        in_=dtile[:, :, :])
                if pk < LAYERS - 1:
                    nc.gpsimd.collective_compute(
                        "AllGather", AL.bypass,
                        replica_groups=[list(range(cfg.n_cores))],
                        ins=[shard[:]], outs=[ufull[pk % 2][:]])

            # final dense matmul: out_part[g, f] = sum_n u3[n, g] * Xp[n, f]
            u3 = fin_pool.tile([128, NBLK, G], f32, tag="u3")
            nc.sync.dma_start(out=u3[:, :, :], in_=shard_pbf)
            fps = ep_pool.tile([G, F], f32, tag="ep")
            for b in range(NBLK):
                xpt = xp_pool.tile([128, F], f32, tag="xp")
                nc.sync.dma_start(out=xpt[:], in_=Xp[b * 128:(b + 1) * 128, :])
                nc.tensor.matmul(fps[:], lhsT=u3[:, b, :], rhs=xpt[:],
                                 start=(b == 0), stop=(b == NBLK - 1))
            outp = fin_pool.tile([G, F], f32, tag="outp")
            nc.vector.tensor_copy(out=outp[:], in_=fps[:])
            nc.sync.dma_start(out=arin[:], in_=outp[:])
            nc.gpsimd.collective_compute(
                "AllReduce", AL.add,
                replica_groups=[list(range(cfg.n_cores))],
                ins=[arin[:]], outs=[arout[:]])
            ar_sb = fin_pool.tile([G, F], f32, tag="arsb")
            nc.sync.dma_start(out=ar_sb[:], in_=arout[:])

            # epilogue: W12 = W1 @ W2 ; W012 = W0 @ W12 ; res^T ; out
            wps = ep_pool.tile([128, F], f32, tag="ep")
            w12 = fin_pool.tile([128, F], f32, tag="w12")
            nc.tensor.matmul(wps[:], lhsT=w_sb[:, F:2 * F], rhs=w_sb[:, 2 * F:3 * F],
                             start=True, stop=True)
            nc.vector.tensor_copy(out=w12[:], in_=wps[:])
            wps2 = ep_pool.tile([128, F], f32, tag="ep")
            w012 = fin_pool.tile([128, F], f32, tag="w012")
            nc.tensor.matmul(wps2[:], lhsT=w_sb[:, 0:F], rhs=w12[:],
                             start=True, stop=True)
            nc.vector.tensor_copy(out=w012[:], in_=wps2[:])
            tps = ep_pool.tile([128, G], f32, tag="ep")
            nc.tensor.transpose(out=tps[:], in_=ar_sb[:, :],
                                identity=ident_sb[:G, :G])
            resT = fin_pool.tile([128, G], f32, tag="resT")
            nc.vector.tensor_copy(out=resT[:], in_=tps[:])
            ops = ep_pool.tile([G, F], f32, tag="ep")
            nc.tensor.matmul(ops[:], lhsT=resT[:], rhs=w012[:], start=True, stop=True)
            icnt = fin_pool.tile([G, 1], f32, tag="icnt")
            nc.sync.dma_start(out=icnt[:], in_=inv_cnt[:])
            fin = fin_pool.tile([G, F], f32, tag="finout")
            nc.vector.tensor_scalar_mul(fin[:], ops[:], icnt[:])
            nc.sync.dma_start(out=out_ext[:], in_=fin[:])

    nc.compile()
    return nc


def make_in_maps(cfg, aux):
    in_maps = []
    for c in range(cfg.n_cores):
        in_maps.append({
            "gidx16": np.ascontiguousarray(aux["gidx16"][c]),
            "nrm_sm": np.ascontiguousarray(aux["nrm_sm"][c]),
            "doff_sm": np.ascontiguousarray(aux["doff_sm"][c]),
            "bg_sm": np.ascontiguousarray(aux["bg_sm"][c]),
            "selfw": np.ascontiguousarray(aux["selfw"][c]),
            "batchloc": np.ascontiguousarray(aux["batchloc"][c]),
            "Xp": np.ascontiguousarray(aux["Xp"][c]),
            "vmaps": np.ascontiguousarray(aux["vmaps"][c]),
            "inv_cnt": aux["inv_cnt"],
            "W0T": aux["W0T"], "W1T": aux["W1T"], "W2": aux["W2"],
        })
    return in_maps


_PROGRAM_CACHE = {}


def kernel(**inputs):
    from concourse.bass_utils import run_bass_kernel_spmd

    cfg = FULL_CFG
    x = np.asarray(inputs["x"], dtype=np.float32)
    edge_index = np.asarray(inputs["edge_index"])
    edge_attr = np.asarray(inputs["edge_attr"], dtype=np.float32)
    batch = np.asarray(inputs["batch"])
    Ws = np.asarray(inputs["Ws"], dtype=np.float32)
    bs = np.asarray(inputs["bs"], dtype=np.float32)
    assert not np.any(bs), "nonzero biases not supported by this kernel build"

    aux = host_prep(cfg, x, edge_index, edge_attr, batch, Ws, bs)
    key = ("full", cfg.slots_total)
    if key not in _PROGRAM_CACHE:
        _PROGRAM_CACHE[key] = build_program(cfg)
    nc = _PROGRAM_CACHE[key]
    in_maps = make_in_maps(cfg, aux)
    res = run_bass_kernel_spmd(nc, in_maps, core_ids=list(range(cfg.n_cores)))
    return np.asarray(res.results[0]["out"], dtype=np.float32)


def emulate(cfg, aux):
    """Emulates the SPMD device program in numpy, including bf16 rounding.

    Mirrors the device structure exactly: per pass, uprev (for self loops)
    comes from the previous pass's *shard* (which includes virtual adds);
    message scaling and one-hots are rounded to bf16 before the PE matmul.
    """
    n_cores = cfg.n_cores
    S = cfg.slots_total
    CR = cfg.core_rows
    s_idx = np.arange(S)

    def slots_of(c, arr):
        return arr[c][s_idx % 128, s_idx // 128]

    # chunk -> (group, psum column block) mapping, in program order
    chunk_blk = {}  # global chunk idx -> (group, block-in-group) or None for pads
    for g, gs in enumerate(cfg.group_sizes):
        for w in range(cfg.n_win):
            off = cfg.gw_slot_off[(g, w)] // 128
            for j in range(gs):
                for sc in range(cfg.seg_chunks):
                    chunk_blk[off + j * cfg.seg_chunks + sc] = (g, j)

    ufull = None
    shard_hist = []
    shards = [np.zeros((CR, G), dtype=np.float32) for _ in range(n_cores)]
    for pk in range(LAYERS):
        prev_shards = [s.copy() for s in shards]
        for c in range(n_cores):
            doff = slots_of(c, aux["doff_sm"]).astype(np.float32)
            nrm = slots_of(c, aux["nrm_sm"])
            if pk == 0:
                bgv = slots_of(c, aux["bg_sm"]).astype(np.float32)
                eq = (bgv[:, None] == np.arange(G)[None, :]).astype(np.float32)
                msg = (eq * nrm[:, None]).astype(BF16)
            else:
                lay = aux["gidx16"][c][:16]
                gi = lay[s_idx % 16, s_idx // 16].astype(np.int64)
                wb = np.zeros(S, dtype=np.int64)
                for (g, w), off in cfg.gw_slot_off.items():
                    n = cfg.gw_chunks[g] * 128
                    wb[off:off + n] = w * cfg.win
                msg = (ufull[gi + wb] * nrm[:, None]).astype(BF16)
            oh = (doff[:, None] == np.arange(128)[None, :]).astype(BF16)

            acc = np.zeros((128, cfg.nblk * G), dtype=np.float32)
            vacc = np.zeros((128, G), dtype=np.float32)
            vg = cfg.n_groups - 1
            gbase = np.cumsum([0] + cfg.group_sizes)
            for cidx in range(S // 128):
                ohc = oh[cidx * 128:(cidx + 1) * 128].astype(np.float32)
                mc = msg[cidx * 128:(cidx + 1) * 128].astype(np.float32)
                if cidx in chunk_blk:
                    g, j = chunk_blk[cidx]
                    b = gbase[g] + j
                    acc[:, b * G:(b + 1) * G] += ohc.T @ mc
                else:
                    vacc += ohc.T @ mc
            # self-loops: uprev from previous shard (or u0)
            if pk == 0:
                bl = aux["batchloc"][c].astype(np.float32)
                uprev = np.zeros((128, cfg.nblk * G), dtype=np.float32)
                for b in range(cfg.nblk):
                    uprev[:, b * G:(b + 1) * G] = (
                        bl[:, b][:, None] == np.arange(G)[None, :])
            else:
                uprev = np.zeros((128, cfg.nblk * G), dtype=np.float32)
                for b in range(cfg.nblk):
                    uprev[:, b * G:(b + 1) * G] = prev_shards[c][b * 128:(b + 1) * 128]
            sw = aux["selfw"][c]
            for b in range(cfg.nblk):
                acc[:, b * G:(b + 1) * G] += sw[:, b][:, None] * uprev[:, b * G:(b + 1) * G]
            # to shard layout + virtual adds
            shard = np.zeros((CR, G), dtype=np.float32)
            for b in range(cfg.nblk):
                shard[b * 128:(b + 1) * 128] = acc[:, b * G:(b + 1) * G]
            vm = aux["vmaps"][c][:, 0]
            for v in range(128):
                if vm[v] <= CR - 1:
                    shard[vm[v]] += vacc[v]
            shards[c] = shard
        shard_hist.append([s.copy() for s in shards])
        if pk < LAYERS - 1:
            ufull = np.concatenate(shards, axis=0)
    out = np.zeros((G, F), dtype=np.float32)
    for c in range(n_cores):
        out += shards[c].T @ aux["Xp"][c]
    W12 = aux["W1T"].T @ aux["W2"]
    W012 = aux["W0T"].T @ W12
    res = out @ W012
    assert not np.any(aux["bs"]), "bias path handled on device only"
    emulate.last_shards = shard_hist
    return res * aux["inv_cnt"]



# revision 3
# speedup vs baseline: 1.6009x; 1.6009x over previous
"""Trainium2 Bass kernel for CellGraphSignatureGNN (GCN message passing).

Math: the network is affine per layer: x_{l+1} = A @ x_l @ W_l + 1 b_l^T,
with A = D^-1/2 (Adj + 2I) D^-1/2 (weighted adjacency + improved self loops),
followed by a per-graph mean pool P (and bs == 0 in this problem).  Since A
acts on nodes and W on features:

    out = P A^3 X (W0 W1 W2) / counts

We evaluate left-to-right: u1^T = A^T P^T, u2^T = A^T u1^T, u3^T = A^T u2^T
(64-wide node vectors), then one dense matmul (u3 X) sharded over nodes, a
tiny AllReduce, and the 128x128 weight chain on-device.

Distribution: nodes are re-labeled and bin-packed into 128-node blocks (100
blocks/core x 8 cores) balanced by scatter-degree so every core runs an
IDENTICAL (SPMD) program; per-core data (gather indices, one-hot offsets,
norms) differ.  Edge (r, c) contributes norm_e * u[c] into u_new[r]:
  - gather u[c] rows (256B fp32) from the window-replicated u in HBM via
    dma_gather (int16 indices), round-robined over all 4 SWDGE queues so all
    8 GpSimd Q7 cores generate DMA descriptors in parallel,
  - scale by norm_e on DVE (bf16 out), build a destination one-hot on DVE,
  - scatter = one-hot matmul accumulated in PSUM per 128-dest block,
  - the per-layer u re-replication is FOUR quarter AllGathers (Shared outputs)
    that pipeline with compute: quarter q's AG is issued as soon as its 25
    blocks are final, and the next pass's window-w work only waits for AG w.
Self-loops are applied as local elementwise ops; rare (block,window) segment
overflow edges go through a "virtual block" + per-quarter indirect
scatter-add with CCE.
"""

import numpy as np
import ml_dtypes

BF16 = ml_dtypes.bfloat16

G = 64        # graphs
F = 128       # feature width
LAYERS = 3
PAD_SENT = 30000.0  # destoff/bg sentinel for padded slots (one-hot -> 0)


# --------------------------------------------------------------------------
# configuration
# --------------------------------------------------------------------------
class Cfg:
    def __init__(self, n_nodes, n_edges, n_cores=8, nblk=100, group_sizes=None,
                 seg_chunks=2, n_win=4, vchunks=2, vcap=128, csub=8):
        self.n_nodes = n_nodes
        self.n_edges = n_edges
        self.n_cores = n_cores
        self.nblk = nblk                      # real blocks per core
        self.group_sizes = group_sizes or [13, 12] * 4
        assert sum(self.group_sizes) == nblk
        self.seg_chunks = seg_chunks          # 128-slot chunks per (block, window)
        self.n_win = n_win
        self.vchunks = vchunks                # chunks per window for the virtual blk
        self.vcap = vcap                      # distinct spill destinations
        self.csub = csub                      # chunks per dma_gather call
        self.core_rows = nblk * 128           # permuted rows per core
        self.pn = n_cores * self.core_rows    # total permuted rows
        assert self.core_rows % n_win == 0
        self.qrows = self.core_rows // n_win  # rows per (core, quarter)
        self.win = n_cores * self.qrows       # rows per assembled window
        assert self.win <= 32768
        assert nblk % n_win == 0
        self.blocks_per_q = nblk // n_win
        # groups must tile quarters exactly: groups [2q], [2q+1] cover quarter q
        assert len(self.group_sizes) == 2 * n_win
        for q in range(n_win):
            assert (self.group_sizes[2 * q] + self.group_sizes[2 * q + 1]
                    == self.blocks_per_q)
        # slot layout: for g in groups + [vgroup]: for w in windows: contig slots
        self.gw_chunks = [gs * seg_chunks for gs in self.group_sizes] + [vchunks]
        self.n_groups = len(self.gw_chunks)   # includes virtual group
        self.slots_total = sum(self.gw_chunks) * 128 * n_win
        self.gw_slot_off = {}
        off = 0
        for g, c in enumerate(self.gw_chunks):
            for w in range(n_win):
                self.gw_slot_off[(g, w)] = off
                off += c * 128
        assert off == self.slots_total


FULL_CFG = Cfg(100000, 640000)


# --------------------------------------------------------------------------
# host-side graph preprocessing (indices, norms, schedules)
# --------------------------------------------------------------------------
def host_prep(cfg, x, edge_index, edge_attr, batch, Ws, bs):
    N, E = cfg.n_nodes, cfg.n_edges
    row = np.asarray(edge_index[0], dtype=np.int64)
    col = np.asarray(edge_index[1], dtype=np.int64)
    w = np.asarray(edge_attr, dtype=np.float32).reshape(-1)
    batch = np.asarray(batch, dtype=np.int64)

    deg = np.zeros(N, dtype=np.float64)
    np.add.at(deg, col, w.astype(np.float64))
    deg += 2.0
    dinv = (1.0 / np.sqrt(deg)).astype(np.float32)
    norm = dinv[row] * w * dinv[col]
    selfnorm = 2.0 * dinv * dinv
    cnt = np.bincount(batch, minlength=G).astype(np.float32)

    # ---- bin-pack nodes into blocks by scatter degree (edges with row == n)
    sdeg = np.bincount(row, minlength=N)
    nbins = cfg.n_cores * cfg.nblk
    order = np.argsort(-sdeg, kind="stable")
    binsum = np.zeros(nbins, dtype=np.int64)
    binfill = np.zeros(nbins, dtype=np.int32)
    import heapq
    heap = [(0, b) for b in range(nbins)]
    heapq.heapify(heap)
    node_bin = np.empty(N, dtype=np.int32)
    node_pos = np.empty(N, dtype=np.int32)
    for n in order:
        while True:
            s, b = heapq.heappop(heap)
            if binfill[b] < 128:
                break
        node_bin[n] = b
        node_pos[n] = binfill[b]
        binfill[b] += 1
        binsum[b] += sdeg[n]
        if binfill[b] < 128:
            heapq.heappush(heap, (int(binsum[b]), b))
    # snake-assign bins to cores by load
    border = np.argsort(-binsum, kind="stable")
    bin_core = np.empty(nbins, dtype=np.int32)
    bin_blk = np.empty(nbins, dtype=np.int32)
    percore = [[] for _ in range(cfg.n_cores)]
    for i, b in enumerate(border):
        r = i // cfg.n_cores
        k = i % cfg.n_cores
        c = k if (r % 2 == 0) else cfg.n_cores - 1 - k
        bin_core[b] = c
        bin_blk[b] = len(percore[c])
        percore[c].append(b)
    assert all(len(p) == cfg.nblk for p in percore)

    # core-local row of a node; window = quarter of the local row
    local_row = bin_blk[node_bin].astype(np.int64) * 128 + node_pos
    node_core = bin_core[node_bin].astype(np.int64)
    node_q = local_row // cfg.qrows
    node_widx = node_core * cfg.qrows + (local_row % cfg.qrows)

    # ---- per-core edge schedules
    e_core = bin_core[node_bin[row]]
    e_blk = bin_blk[node_bin[row]]
    e_doff = node_pos[row]                 # dest offset within block
    e_win = node_q[col].astype(np.int32)   # source window (quarter)
    e_gidx = node_widx[col].astype(np.int32)  # gather idx within window

    SEG = cfg.seg_chunks * 128
    S = cfg.slots_total
    n_cores = cfg.n_cores

    gidx = np.zeros((n_cores, S), dtype=np.int32)
    doff = np.full((n_cores, S), PAD_SENT, dtype=np.float32)
    nrm = np.zeros((n_cores, S), dtype=np.float32)
    bg = np.full((n_cores, S), PAD_SENT, dtype=np.float32)

    # block -> (group, block-in-group) mapping
    blk_group = []
    blk_ing = []
    for g, gs in enumerate(cfg.group_sizes):
        for j in range(gs):
            blk_group.append(g)
            blk_ing.append(j)
    blk_group = np.array(blk_group)
    blk_ing = np.array(blk_ing)

    # per-quarter rebased virtual spill maps (1<<30 = inactive / out of range)
    vmaps = np.full((n_cores, 128, cfg.n_win), 1 << 30, dtype=np.int32)
    spill_warn = 0

    for c in range(n_cores):
        em = e_core == c
        eb = e_blk[em]
        ew = e_win[em]
        eg = e_gidx[em]
        ed = e_doff[em]
        en = norm[em]
        ebg = batch[col[em]].astype(np.float32)
        # order edges by (block, window)
        key = eb * cfg.n_win + ew
        o = np.argsort(key, kind="stable")
        eb, ew, eg, ed, en, ebg = eb[o], ew[o], eg[o], ed[o], en[o], ebg[o]
        spill_list = []
        kk = eb * cfg.n_win + ew
        bounds = np.searchsorted(kk, np.arange(cfg.nblk * cfg.n_win + 1))
        vused = {}
        for b in range(cfg.nblk):
            g = blk_group[b]
            j = blk_ing[b]
            for wi in range(cfg.n_win):
                lo, hi = bounds[b * cfg.n_win + wi], bounds[b * cfg.n_win + wi + 1]
                nseg = hi - lo
                take = min(nseg, SEG)
                base = (cfg.gw_slot_off[(g, wi)] + j * SEG)
                sl = slice(base, base + take)
                gidx[c, sl] = eg[lo:lo + take]
                doff[c, sl] = ed[lo:lo + take]
                nrm[c, sl] = en[lo:lo + take]
                bg[c, sl] = ebg[lo:lo + take]
                for t in range(lo + take, hi):
                    spill_list.append((b, ew[t], eg[t], ed[t], en[t], ebg[t]))
        # spills into virtual group
        vg = cfg.n_groups - 1
        vfill = np.zeros(cfg.n_win, dtype=np.int32)
        for (b, wi, gg, dd, nn, bb) in spill_list:
            key2 = (b, dd)
            if key2 not in vused:
                assert len(vused) < cfg.vcap, "virtual dest capacity exceeded"
                v = len(vused)
                vused[key2] = v
                r = b * 128 + dd
                q = r // cfg.qrows
                vmaps[c, v, q] = r - q * cfg.qrows
            v = vused[key2]
            assert vfill[wi] < cfg.vchunks * 128, "virtual slot capacity exceeded"
            base = cfg.gw_slot_off[(vg, wi)] + vfill[wi]
            gidx[c, base] = gg
            doff[c, base] = v
            nrm[c, base] = nn
            bg[c, base] = bb
            vfill[wi] += 1
        spill_warn += len(spill_list)

    # ---- pack aux arrays
    ncol16 = S // 16
    gidx16 = np.zeros((n_cores, 128, ncol16), dtype=np.int16)
    s_idx = np.arange(S)
    for c in range(n_cores):
        lay = np.zeros((16, ncol16), dtype=np.int16)
        lay[s_idx % 16, s_idx // 16] = gidx[c].astype(np.int16)
        gidx16[c] = np.tile(lay, (8, 1))
    ncol128 = S // 128

    def slotmajor(a, dt):
        out = np.zeros((n_cores, 128, ncol128), dtype=dt)
        for c in range(n_cores):
            out[c][s_idx % 128, s_idx // 128] = a[c]
        return out

    nrm_sm = slotmajor(nrm, np.float32)
    doff_sm = slotmajor(doff, BF16)
    bg_sm = slotmajor(bg, BF16)

    # ---- per-core node-level aux
    selfw = np.zeros((n_cores, 128, cfg.nblk), dtype=np.float32)
    batchloc = np.full((n_cores, 128, cfg.nblk), PAD_SENT, dtype=BF16)
    Xp = np.zeros((n_cores, cfg.core_rows, F), dtype=np.float32)
    nodes = np.arange(N)
    pc = bin_core[node_bin]
    pb = bin_blk[node_bin]
    pp = node_pos
    for c in range(n_cores):
        m = pc == c
        selfw[c][pp[m], pb[m]] = selfnorm[nodes[m]]
        batchloc[c][pp[m], pb[m]] = batch[nodes[m]].astype(np.float32)
        Xp[c][pb[m] * 128 + pp[m]] = np.asarray(x, dtype=np.float32)[nodes[m]]

    inv_cnt = (1.0 / np.maximum(cnt, 1.0)).astype(np.float32).reshape(G, 1)
    Ws = np.asarray(Ws, dtype=np.float32)
    bs = np.asarray(bs, dtype=np.float32)

    aux = dict(
        gidx16=gidx16, nrm_sm=nrm_sm, doff_sm=doff_sm, bg_sm=bg_sm,
        selfw=selfw, batchloc=batchloc, Xp=Xp, vmaps=vmaps,
        inv_cnt=inv_cnt,
        W0T=np.ascontiguousarray(Ws[0].T), W1T=np.ascontiguousarray(Ws[1].T),
        W2=np.ascontiguousarray(Ws[2]), bs=bs,
        spills=spill_warn,
    )
    return aux


def _midbcast(ap, count):
    """Insert a step-0 middle axis: [P, X] -> [P, (0,count), X]."""
    import concourse.bass as bass
    assert len(ap.ap) == 2
    return bass.AP(ap.tensor, ap.offset, [ap.ap[0], [0, count], ap.ap[1]])


def build_program(cfg, dbg=False):
    import contextlib
    import concourse.bacc as bacc
    import concourse.bass as bass
    import concourse.mybir as mybir
    import concourse.tile as tile

    f32 = mybir.dt.float32
    bf16 = mybir.dt.bfloat16
    i16 = mybir.dt.int16
    i32 = mybir.dt.int32
    AL = mybir.AluOpType

    S = cfg.slots_total
    NBLK = cfg.nblk
    NW = cfg.n_win
    SEGC = cfg.seg_chunks
    BPQ = cfg.blocks_per_q
    QR = cfg.qrows
    gbase = [0]
    for gs in cfg.group_sizes:
        gbase.append(gbase[-1] + gs)

    nc = bacc.Bacc("TRN2", debug=False, num_devices=cfg.n_cores,
                   num_swdge_queues=4)
    P = nc.declare_dram_parameter

    gidx16 = P("gidx16", [128, S // 16], i16, isOutput=False)
    nrm_sm = P("nrm_sm", [128, S // 128], f32, isOutput=False)
    doff_sm = P("doff_sm", [128, S // 128], bf16, isOutput=False)
    bg_sm = P("bg_sm", [128, S // 128], bf16, isOutput=False)
    selfw = P("selfw", [128, NBLK], f32, isOutput=False)
    batchloc = P("batchloc", [128, NBLK], bf16, isOutput=False)
    Xp = P("Xp", [cfg.core_rows, F], f32, isOutput=False)
    vmaps = P("vmaps", [128, NW], i32, isOutput=False)
    inv_cnt = P("inv_cnt", [G, 1], f32, isOutput=False)
    W0T = P("W0T", [F, F], f32, isOutput=False)
    W1T = P("W1T", [F, F], f32, isOutput=False)
    W2 = P("W2", [F, F], f32, isOutput=False)
    out_ext = P("out", [G, F], f32, isOutput=True)

    # per-quarter shard (AG input / final u3) and window-assembled u (AG out)
    shard_q = [nc.dram_tensor(f"shard_q{q}", [QR, G], f32) for q in range(NW)]
    uq = [[nc.dram_tensor(f"u{p}_q{q}", [cfg.win, G], f32)
           for q in range(NW)] for p in range(2)]
    arin = nc.dram_tensor("arin", [G, F], f32)
    arout = nc.dram_tensor("arout", [G, F], f32)

    iota128_c = nc.inline_tensor(
        np.tile(np.arange(128, dtype=np.float32).astype(BF16), (128, 1)), "iota128")
    iota64_c = nc.inline_tensor(
        np.tile(np.arange(G, dtype=np.float32).astype(BF16), (128, 1)), "iota64")
    ident_c = nc.inline_tensor(np.eye(128, dtype=np.float32), "ident")

    qn = [0]

    def next_q():
        qn[0] = (qn[0] + 1) % 4
        return qn[0]

    with tile.TileContext(nc) as tc:
        with contextlib.ExitStack() as ctx:
            perm_pool = ctx.enter_context(tc.tile_pool(name="perm", bufs=1))
            acc = perm_pool.tile([128, NBLK, G], f32, tag="acc")
            uprev = perm_pool.tile([128, NBLK, G], f32, tag="uprev")
            selfw_sb = perm_pool.tile([128, NBLK], f32, tag="selfw")
            io128 = perm_pool.tile([128, 128], bf16, tag="io128")
            io64 = perm_pool.tile([128, G], bf16, tag="io64")
            ident_sb = perm_pool.tile([128, 128], f32, tag="ident")
            vmap_sb = perm_pool.tile([128, NW], i32, tag="vmap")
            w_sb = perm_pool.tile([128, 3 * F], f32, tag="wsb")

            nc.sync.dma_start(out=selfw_sb[:], in_=selfw[:])
            nc.sync.dma_start(out=io128[:], in_=iota128_c[:])
            nc.sync.dma_start(out=io64[:], in_=iota64_c[:])
            nc.sync.dma_start(out=ident_sb[:], in_=ident_c[:])
            nc.sync.dma_start(out=vmap_sb[:], in_=vmaps[:])
            nc.sync.dma_start(out=w_sb[:, 0:F], in_=W0T[:])
            nc.sync.dma_start(out=w_sb[:, F:2 * F], in_=W1T[:])
            nc.sync.dma_start(out=w_sb[:, 2 * F:3 * F], in_=W2[:])

            idx_pool = ctx.enter_context(tc.tile_pool(name="idx", bufs=4))
            aux_pool = ctx.enter_context(tc.tile_pool(name="aux", bufs=4))
            raw_pool = ctx.enter_context(tc.tile_pool(name="raw", bufs=4))
            msg_pool = ctx.enter_context(tc.tile_pool(name="msg", bufs=4))
            oh_pool = ctx.enter_context(tc.tile_pool(name="oh", bufs=4))
            ps_pool = ctx.enter_context(tc.tile_pool(name="ps", bufs=2, space="PSUM"))
            vps_pool = ctx.enter_context(tc.tile_pool(name="vps", bufs=2,
                                                      space="PSUM"))
            ep_pool = ctx.enter_context(tc.tile_pool(name="ep", bufs=2, space="PSUM"))
            fin_pool = ctx.enter_context(tc.tile_pool(name="fin", bufs=2))
            xp_pool = ctx.enter_context(tc.tile_pool(name="xp", bufs=16))

            # [QR, G] viewed as [128p, BPQ, G]
            shard_pbf = [shard_q[q][:].rearrange("(b p) f -> p b f", p=128)
                         for q in range(NW)]
            max_gs = max(cfg.group_sizes)

            def build_msgs(pk, g, w, C, soff):
                """Returns (msgt, oht) for the (g, w) slot region."""
                coff = soff // 128
                nrmt = aux_pool.tile([128, C], f32, tag="nrm")
                nc.sync.dma_start(out=nrmt[:], in_=nrm_sm[:, coff:coff + C])
                dofft = aux_pool.tile([128, C], bf16, tag="doff")
                nc.sync.dma_start(out=dofft[:], in_=doff_sm[:, coff:coff + C])
                msgt = msg_pool.tile([128, C, G], bf16, tag="msg")
                if pk == 0:
                    bgt = aux_pool.tile([128, C], bf16, tag="bg")
                    nc.sync.dma_start(out=bgt[:], in_=bg_sm[:, coff:coff + C])
                    nrm16 = aux_pool.tile([128, C], bf16, tag="nrm16")
                    nc.vector.tensor_copy(out=nrm16[:], in_=nrmt[:])
                    eqt = raw_pool.tile([128, C, G], bf16, tag="raw16")
                    nc.vector.tensor_tensor(
                        out=eqt[:, :, :],
                        in0=bgt[:].to_broadcast([128, C, G]),
                        in1=_midbcast(io64[:], C),
                        op=AL.is_equal)
                    nc.vector.tensor_tensor(
                        out=msgt[:, :, :], in0=eqt[:, :, :],
                        in1=nrm16[:].to_broadcast([128, C, G]),
                        op=AL.mult)
                else:
                    src = uq[(pk + 1) % 2][w]
                    idxt = idx_pool.tile([128, C * 8], i16, tag="idx")
                    nc.sync.dma_start(
                        out=idxt[:],
                        in_=gidx16[:, soff // 16:soff // 16 + C * 8])
                    rawt = raw_pool.tile([128, C, G], f32, tag="raw")
                    for sub in range(0, C, cfg.csub):
                        cs = min(cfg.csub, C - sub)
                        nc.gpsimd.dma_gather(
                            rawt[:, sub:sub + cs, :],
                            src[:, :],
                            idxt[:, sub * 8:(sub + cs) * 8],
                            cs * 128, cs * 128, G,
                            single_packet=False,
                            queue_num=next_q())
                    nc.vector.tensor_tensor(
                        out=msgt[:, :, :], in0=rawt[:, :, :],
                        in1=nrmt[:].to_broadcast([128, C, G]),
                        op=AL.mult)
                oht = oh_pool.tile([128, C, 128], bf16, tag="oh")
                nc.vector.tensor_tensor(
                    out=oht[:, :, :],
                    in0=dofft[:].to_broadcast([128, C, 128]),
                    in1=_midbcast(io128[:], C),
                    op=AL.is_equal)
                return msgt, oht

            def do_pass(pk):
                # self-loop source
                if pk == 0:
                    blc = aux_pool.tile([128, NBLK], bf16, tag="blc")
                    nc.sync.dma_start(out=blc[:], in_=batchloc[:])
                    nc.vector.tensor_tensor(
                        out=uprev[:, :, :],
                        in0=blc[:].to_broadcast([128, NBLK, G]),
                        in1=_midbcast(io64[:], NBLK),
                        op=AL.is_equal)
                else:
                    for q in range(NW):
                        nc.sync.dma_start(
                            out=uprev[:, q * BPQ:(q + 1) * BPQ, :],
                            in_=shard_pbf[q])

                # virtual (spill) group first so vsb is ready per quarter
                vg = cfg.n_groups - 1
                VC = cfg.vchunks
                vsb = fin_pool.tile([128, G], f32, tag="vsb")
                for w in range(NW):
                    soff = cfg.gw_slot_off[(vg, w)]
                    msgt, oht = build_msgs(pk, vg, w, VC, soff)
                    vps = vps_pool.tile([128, G], f32, tag="vps")
                    for ci in range(VC):
                        nc.tensor.matmul(
                            vps[:, :], lhsT=oht[:, ci, :], rhs=msgt[:, ci, :],
                            start=(ci == 0), stop=(ci == VC - 1))
                    if w == 0:
                        nc.vector.tensor_copy(out=vsb[:], in_=vps[:])
                    else:
                        nc.vector.tensor_tensor(
                            out=vsb[:], in0=vsb[:], in1=vps[:], op=AL.add)

                for qo in range(NW):
                    for g in (2 * qo, 2 * qo + 1):
                        gs = cfg.group_sizes[g]
                        C = cfg.gw_chunks[g]
                        seg = slice(gbase[g], gbase[g] + gs)
                        nc.vector.tensor_tensor(
                            out=acc[:, seg, :], in0=uprev[:, seg, :],
                            in1=selfw_sb[:, seg].to_broadcast([128, gs, G]),
                            op=AL.mult)
                        for w in range(NW):
                            soff = cfg.gw_slot_off[(g, w)]
                            msgt, oht = build_msgs(pk, g, w, C, soff)
                            ps = ps_pool.tile([128, max_gs * G], f32, tag="ps")
                            for ci in range(C):
                                j = ci // SEGC
                                nc.tensor.matmul(
                                    ps[:, j * G:(j + 1) * G],
                                    lhsT=oht[:, ci, :], rhs=msgt[:, ci, :],
                                    start=(ci % SEGC == 0),
                                    stop=(ci % SEGC == SEGC - 1))
                            nc.vector.tensor_tensor(
                                out=acc[:, seg, :], in0=acc[:, seg, :],
                                in1=ps[:, :gs * G].rearrange(
                                    "p (b f) -> p b f", f=G),
                                op=AL.add)
                    # quarter qo finalized: write shard, add spills, AllGather
                    nc.sync.dma_start(
                        out=shard_pbf[qo],
                        in_=acc[:, qo * BPQ:(qo + 1) * BPQ, :])
                    nc.gpsimd.indirect_dma_start(
                        out=shard_q[qo][:, :],
                        out_offset=bass.IndirectOffsetOnAxis(
                            ap=vmap_sb[:, qo:qo + 1], axis=0),
                        in_=vsb[:, :], in_offset=None,
                        bounds_check=QR - 1, oob_is_err=False,
                        compute_op=AL.add)
                    if pk < LAYERS - 1:
                        nc.gpsimd.collective_compute(
                            "AllGather", AL.bypass,
                            replica_groups=[list(range(cfg.n_cores))],
                            ins=[shard_q[qo][:]], outs=[uq[pk % 2][qo][:]])

            for pk in range(LAYERS):
                do_pass(pk)

            # final dense matmul: out_part[g, f] = sum_n u3[n, g] * Xp[n, f]
            fps = ep_pool.tile([G, F], f32, tag="ep")
            for qo in range(NW):
                u3 = fin_pool.tile([128, BPQ, G], f32, tag="u3")
                nc.sync.dma_start(out=u3[:, :, :], in_=shard_pbf[qo])
                for b in range(BPQ):
                    gb = qo * BPQ + b
                    xpt = xp_pool.tile([128, F], f32, tag="xp")
                    nc.sync.dma_start(out=xpt[:],
                                      in_=Xp[gb * 128:(gb + 1) * 128, :])
                    nc.tensor.matmul(fps[:], lhsT=u3[:, b, :], rhs=xpt[:],
                                     start=(gb == 0), stop=(gb == NBLK - 1))
            outp = fin_pool.tile([G, F], f32, tag="outp")
            nc.vector.tensor_copy(out=outp[:], in_=fps[:])
            nc.sync.dma_start(out=arin[:], in_=outp[:])
            nc.gpsimd.collective_compute(
                "AllReduce", AL.add,
                replica_groups=[list(range(cfg.n_cores))],
                ins=[arin[:]], outs=[arout[:]])
            ar_sb = fin_pool.tile([G, F], f32, tag="arsb")
            nc.sync.dma_start(out=ar_sb[:], in_=arout[:])

            # epilogue: W12 = W1 @ W2 ; W012 = W0 @ W12 ; res^T ; out
            wps = ep_pool.tile([128, F], f32, tag="ep")
            w12 = fin_pool.tile([128, F], f32, tag="w12")
            nc.tensor.matmul(wps[:], lhsT=w_sb[:, F:2 * F], rhs=w_sb[:, 2 * F:3 * F],
                             start=True, stop=True)
            nc.vector.tensor_copy(out=w12[:], in_=wps[:])
            wps2 = ep_pool.tile([128, F], f32, tag="ep")
            w012 = fin_pool.tile([128, F], f32, tag="w012")
            nc.tensor.matmul(wps2[:], lhsT=w_sb[:, 0:F], rhs=w12[:],
                             start=True, stop=True)
            nc.vector.tensor_copy(out=w012[:], in_=wps2[:])
            tps = ep_pool.tile([128, G], f32, tag="ep")
            nc.tensor.transpose(out=tps[:], in_=ar_sb[:, :],
                                identity=ident_sb[:G, :G])
            resT = fin_pool.tile([128, G], f32, tag="resT")
            nc.vector.tensor_copy(out=resT[:], in_=tps[:])
            ops = ep_pool.tile([G, F], f32, tag="ep")
            nc.tensor.matmul(ops[:], lhsT=resT[:], rhs=w012[:], start=True, stop=True)
            icnt = fin_pool.tile([G, 1], f32, tag="icnt")
            nc.sync.dma_start(out=icnt[:], in_=inv_cnt[:])
            fin = fin_pool.tile([G, F], f32, tag="finout")
            nc.vector.tensor_scalar_mul(fin[:], ops[:], icnt[:])
            nc.sync.dma_start(out=out_ext[:], in_=fin[:])

    nc.compile()
    return nc


def make_in_maps(cfg, aux):
    in_maps = []
    for c in range(cfg.n_cores):
        in_maps.append({
            "gidx16": np.ascontiguousarray(aux["gidx16"][c]),
            "nrm_sm": np.ascontiguousarray(aux["nrm_sm"][c]),
            "doff_sm": np.ascontiguousarray(aux["doff_sm"][c]),
            "bg_sm": np.ascontiguousarray(aux["bg_sm"][c]),
            "selfw": np.ascontiguousarray(aux["selfw"][c]),
            "batchloc": np.ascontiguousarray(aux["batchloc"][c]),
            "Xp": np.ascontiguousarray(aux["Xp"][c]),
            "vmaps": np.ascontiguousarray(aux["vmaps"][c]),
            "inv_cnt": aux["inv_cnt"],
            "W0T": aux["W0T"], "W1T": aux["W1T"], "W2": aux["W2"],
        })
    return in_maps


_PROGRAM_CACHE = {}


def kernel(**inputs):
    from concourse.bass_utils import run_bass_kernel_spmd

    cfg = FULL_CFG
    x = np.asarray(inputs["x"], dtype=np.float32)
    edge_index = np.asarray(inputs["edge_index"])
    edge_attr = np.asarray(inputs["edge_attr"], dtype=np.float32)
    batch = np.asarray(inputs["batch"])
    Ws = np.asarray(inputs["Ws"], dtype=np.float32)
    bs = np.asarray(inputs["bs"], dtype=np.float32)
    assert not np.any(bs), "nonzero biases not supported by this kernel build"

    aux = host_prep(cfg, x, edge_index, edge_attr, batch, Ws, bs)
    key = ("full", cfg.slots_total)
    if key not in _PROGRAM_CACHE:
        _PROGRAM_CACHE[key] = build_program(cfg)
    nc = _PROGRAM_CACHE[key]
    in_maps = make_in_maps(cfg, aux)
    res = run_bass_kernel_spmd(nc, in_maps, core_ids=list(range(cfg.n_cores)))
    return np.asarray(res.results[0]["out"], dtype=np.float32)


# revision 10
# speedup vs baseline: 1.6210x; 1.0125x over previous
"""Trainium2 Bass kernel for CellGraphSignatureGNN (GCN message passing).

Math: the network is affine per layer: x_{l+1} = A @ x_l @ W_l + 1 b_l^T,
with A = D^-1/2 (Adj + 2I) D^-1/2 (weighted adjacency + improved self loops),
followed by a per-graph mean pool P (and bs == 0 in this problem).  Since A
acts on nodes and W on features:

    out = P A^3 X (W0 W1 W2) / counts

We evaluate left-to-right: u1^T = A^T P^T, u2^T = A^T u1^T, u3^T = A^T u2^T
(64-wide node vectors), then one dense matmul (u3 X) sharded over nodes, a
tiny AllReduce, and the 128x128 weight chain on-device.

Distribution: nodes are re-labeled and bin-packed into 128-node blocks (100
blocks/core x 8 cores) balanced by scatter-degree so every core runs an
IDENTICAL (SPMD) program; per-core data (gather indices, one-hot offsets,
norms) differ.  Edge (r, c) contributes norm_e * u[c] into u_new[r]:
  - gather u[c] rows (256B fp32) from the window-replicated u in HBM via
    dma_gather (int16 indices), round-robined over all 4 SWDGE queues so all
    8 GpSimd Q7 cores generate DMA descriptors in parallel,
  - scale by norm_e on DVE (bf16 out), build a destination one-hot on DVE,
  - scatter = one-hot matmul accumulated in PSUM per 128-dest block,
  - the per-layer u re-replication is FOUR quarter AllGathers (Shared outputs)
    that pipeline with compute: quarter q's AG is issued as soon as its 25
    blocks are final, and the next pass's window-w work only waits for AG w.
Self-loops are applied as local elementwise ops; rare (block,window) segment
overflow edges go through a "virtual block" + per-quarter indirect
scatter-add with CCE.
"""

import numpy as np
import ml_dtypes

BF16 = ml_dtypes.bfloat16

G = 64        # graphs
F = 128       # feature width
LAYERS = 3
PAD_SENT = 30000.0  # destoff/bg sentinel for padded slots (one-hot -> 0)


# --------------------------------------------------------------------------
# configuration
# --------------------------------------------------------------------------
class Cfg:
    def __init__(self, n_nodes, n_edges, n_cores=8, nblk=100, group_sizes=None,
                 seg_chunks=2, n_win=4, vchunks=2, vcap=128, csub=8):
        self.n_nodes = n_nodes
        self.n_edges = n_edges
        self.n_cores = n_cores
        self.nblk = nblk                      # real blocks per core
        self.group_sizes = group_sizes or [13, 12] * 4
        assert sum(self.group_sizes) == nblk
        self.seg_chunks = seg_chunks          # 128-slot chunks per (block, window)
        self.n_win = n_win
        self.vchunks = vchunks                # chunks per window for the virtual blk
        self.vcap = vcap                      # distinct spill destinations
        self.csub = csub                      # chunks per dma_gather call
        self.core_rows = nblk * 128           # permuted rows per core
        self.pn = n_cores * self.core_rows    # total permuted rows
        assert self.core_rows % n_win == 0
        self.qrows = self.core_rows // n_win  # rows per (core, quarter)
        self.win = n_cores * self.qrows       # rows per assembled window
        assert self.win <= 32768
        assert nblk % n_win == 0
        self.blocks_per_q = nblk // n_win
        # groups must tile quarters exactly: groups [2q], [2q+1] cover quarter q
        assert len(self.group_sizes) == 2 * n_win
        for q in range(n_win):
            assert (self.group_sizes[2 * q] + self.group_sizes[2 * q + 1]
                    == self.blocks_per_q)
        # slot layout: for g in groups + [vgroup]: for w in windows: contig slots
        self.gw_chunks = [gs * seg_chunks for gs in self.group_sizes] + [vchunks]
        self.n_groups = len(self.gw_chunks)   # includes virtual group
        self.slots_total = sum(self.gw_chunks) * 128 * n_win
        self.gw_slot_off = {}
        off = 0
        for g, c in enumerate(self.gw_chunks):
            for w in range(n_win):
                self.gw_slot_off[(g, w)] = off
                off += c * 128
        assert off == self.slots_total


FULL_CFG = Cfg(100000, 640000)


# --------------------------------------------------------------------------
# host-side graph preprocessing (indices, norms, schedules)
# --------------------------------------------------------------------------
def host_prep(cfg, x, edge_index, edge_attr, batch, Ws, bs):
    N, E = cfg.n_nodes, cfg.n_edges
    row = np.asarray(edge_index[0], dtype=np.int64)
    col = np.asarray(edge_index[1], dtype=np.int64)
    w = np.asarray(edge_attr, dtype=np.float32).reshape(-1)
    batch = np.asarray(batch, dtype=np.int64)

    deg = np.zeros(N, dtype=np.float64)
    np.add.at(deg, col, w.astype(np.float64))
    deg += 2.0
    dinv = (1.0 / np.sqrt(deg)).astype(np.float32)
    norm = dinv[row] * w * dinv[col]
    selfnorm = 2.0 * dinv * dinv
    cnt = np.bincount(batch, minlength=G).astype(np.float32)

    # ---- bin-pack nodes into blocks by scatter degree (edges with row == n)
    sdeg = np.bincount(row, minlength=N)
    nbins = cfg.n_cores * cfg.nblk
    order = np.argsort(-sdeg, kind="stable")
    binsum = np.zeros(nbins, dtype=np.int64)
    binfill = np.zeros(nbins, dtype=np.int32)
    import heapq
    heap = [(0, b) for b in range(nbins)]
    heapq.heapify(heap)
    node_bin = np.empty(N, dtype=np.int32)
    node_pos = np.empty(N, dtype=np.int32)
    for n in order:
        while True:
            s, b = heapq.heappop(heap)
            if binfill[b] < 128:
                break
        node_bin[n] = b
        node_pos[n] = binfill[b]
        binfill[b] += 1
        binsum[b] += sdeg[n]
        if binfill[b] < 128:
            heapq.heappush(heap, (int(binsum[b]), b))
    # snake-assign bins to cores by load
    border = np.argsort(-binsum, kind="stable")
    bin_core = np.empty(nbins, dtype=np.int32)
    bin_blk = np.empty(nbins, dtype=np.int32)
    percore = [[] for _ in range(cfg.n_cores)]
    for i, b in enumerate(border):
        r = i // cfg.n_cores
        k = i % cfg.n_cores
        c = k if (r % 2 == 0) else cfg.n_cores - 1 - k
        bin_core[b] = c
        bin_blk[b] = len(percore[c])
        percore[c].append(b)
    assert all(len(p) == cfg.nblk for p in percore)

    # core-local row of a node; window = quarter of the local row
    local_row = bin_blk[node_bin].astype(np.int64) * 128 + node_pos
    node_core = bin_core[node_bin].astype(np.int64)
    node_q = local_row // cfg.qrows
    node_widx = node_core * cfg.qrows + (local_row % cfg.qrows)

    # ---- per-core edge schedules
    e_core = bin_core[node_bin[row]]
    e_blk = bin_blk[node_bin[row]]
    e_doff = node_pos[row]                 # dest offset within block
    e_win = node_q[col].astype(np.int32)   # source window (quarter)
    e_gidx = node_widx[col].astype(np.int32)  # gather idx within window

    SEG = cfg.seg_chunks * 128
    S = cfg.slots_total
    n_cores = cfg.n_cores

    gidx = np.zeros((n_cores, S), dtype=np.int32)
    doff = np.full((n_cores, S), PAD_SENT, dtype=np.float32)
    nrm = np.zeros((n_cores, S), dtype=np.float32)
    bg = np.full((n_cores, S), PAD_SENT, dtype=np.float32)

    # block -> (group, block-in-group) mapping
    blk_group = []
    blk_ing = []
    for g, gs in enumerate(cfg.group_sizes):
        for j in range(gs):
            blk_group.append(g)
            blk_ing.append(j)
    blk_group = np.array(blk_group)
    blk_ing = np.array(blk_ing)

    # per-quarter rebased virtual spill maps (1<<30 = inactive / out of range)
    vmaps = np.full((n_cores, 128, cfg.n_win), 1 << 30, dtype=np.int32)
    spill_warn = 0

    for c in range(n_cores):
        em = e_core == c
        eb = e_blk[em]
        ew = e_win[em]
        eg = e_gidx[em]
        ed = e_doff[em]
        en = norm[em]
        ebg = batch[col[em]].astype(np.float32)
        # order edges by (block, window)
        key = eb * cfg.n_win + ew
        o = np.argsort(key, kind="stable")
        eb, ew, eg, ed, en, ebg = eb[o], ew[o], eg[o], ed[o], en[o], ebg[o]
        spill_list = []
        kk = eb * cfg.n_win + ew
        bounds = np.searchsorted(kk, np.arange(cfg.nblk * cfg.n_win + 1))
        vused = {}
        for b in range(cfg.nblk):
            g = blk_group[b]
            j = blk_ing[b]
            for wi in range(cfg.n_win):
                lo, hi = bounds[b * cfg.n_win + wi], bounds[b * cfg.n_win + wi + 1]
                nseg = hi - lo
                take = min(nseg, SEG)
                base = (cfg.gw_slot_off[(g, wi)] + j * SEG)
                sl = slice(base, base + take)
                gidx[c, sl] = eg[lo:lo + take]
                doff[c, sl] = ed[lo:lo + take]
                nrm[c, sl] = en[lo:lo + take]
                bg[c, sl] = ebg[lo:lo + take]
                for t in range(lo + take, hi):
                    spill_list.append((b, ew[t], eg[t], ed[t], en[t], ebg[t]))
        # spills into virtual group
        vg = cfg.n_groups - 1
        vfill = np.zeros(cfg.n_win, dtype=np.int32)
        for (b, wi, gg, dd, nn, bb) in spill_list:
            key2 = (b, dd)
            if key2 not in vused:
                assert len(vused) < cfg.vcap, "virtual dest capacity exceeded"
                v = len(vused)
                vused[key2] = v
                r = b * 128 + dd
                q = r // cfg.qrows
                vmaps[c, v, q] = r - q * cfg.qrows
            v = vused[key2]
            assert vfill[wi] < cfg.vchunks * 128, "virtual slot capacity exceeded"
            base = cfg.gw_slot_off[(vg, wi)] + vfill[wi]
            gidx[c, base] = gg
            doff[c, base] = v
            nrm[c, base] = nn
            bg[c, base] = bb
            vfill[wi] += 1
        spill_warn += len(spill_list)

    # ---- pack aux arrays
    ncol16 = S // 16
    gidx16 = np.zeros((n_cores, 128, ncol16), dtype=np.int16)
    s_idx = np.arange(S)
    for c in range(n_cores):
        lay = np.zeros((16, ncol16), dtype=np.int16)
        lay[s_idx % 16, s_idx // 16] = gidx[c].astype(np.int16)
        gidx16[c] = np.tile(lay, (8, 1))
    ncol128 = S // 128

    def slotmajor(a, dt):
        out = np.zeros((n_cores, 128, ncol128), dtype=dt)
        for c in range(n_cores):
            out[c][s_idx % 128, s_idx // 128] = a[c]
        return out

    nrm_sm = slotmajor(nrm, np.float32)
    doff_sm = slotmajor(doff, BF16)
    bg_sm = slotmajor(bg, BF16)

    # host-prebuilt destination one-hots (all passes) and pass-0 messages
    SC = S // 128
    p_i = s_idx % 128
    c_i = s_idx // 128
    oh_hbm = np.zeros((n_cores, 128, SC, 128), dtype=BF16)
    msg0_hbm = np.zeros((n_cores, 128, SC, G), dtype=BF16)
    for c in range(n_cores):
        dv = doff[c].astype(np.int64)
        m = doff[c] != PAD_SENT
        oh_hbm[c][p_i[m], c_i[m], dv[m]] = 1
        bv = bg[c].astype(np.int64)
        mb = bg[c] != PAD_SENT
        msg0_hbm[c][p_i[mb], c_i[mb], bv[mb]] = nrm[c][mb].astype(BF16)

    # ---- per-core node-level aux
    selfw = np.zeros((n_cores, 128, cfg.nblk), dtype=np.float32)
    batchloc = np.full((n_cores, 128, cfg.nblk), PAD_SENT, dtype=BF16)
    Xp = np.zeros((n_cores, cfg.core_rows, F), dtype=np.float32)
    nodes = np.arange(N)
    pc = bin_core[node_bin]
    pb = bin_blk[node_bin]
    pp = node_pos
    for c in range(n_cores):
        m = pc == c
        selfw[c][pp[m], pb[m]] = selfnorm[nodes[m]]
        batchloc[c][pp[m], pb[m]] = batch[nodes[m]].astype(np.float32)
        Xp[c][pb[m] * 128 + pp[m]] = np.asarray(x, dtype=np.float32)[nodes[m]]

    inv_cnt = (1.0 / np.maximum(cnt, 1.0)).astype(np.float32).reshape(G, 1)
    Ws = np.asarray(Ws, dtype=np.float32)
    bs = np.asarray(bs, dtype=np.float32)

    aux = dict(
        gidx16=gidx16, nrm_sm=nrm_sm, doff_sm=doff_sm, bg_sm=bg_sm,
        oh_hbm=oh_hbm, msg0_hbm=msg0_hbm,
        selfw=selfw, batchloc=batchloc, Xp=Xp, vmaps=vmaps,
        inv_cnt=inv_cnt,
        W0T=np.ascontiguousarray(Ws[0].T), W1T=np.ascontiguousarray(Ws[1].T),
        W2=np.ascontiguousarray(Ws[2]), bs=bs,
        spills=spill_warn,
    )
    return aux


def _midbcast(ap, count):
    """Insert a step-0 middle axis: [P, X] -> [P, (0,count), X]."""
    import concourse.bass as bass
    assert len(ap.ap) == 2
    return bass.AP(ap.tensor, ap.offset, [ap.ap[0], [0, count], ap.ap[1]])


def build_program(cfg, use_virtual=True):
    import contextlib
    import concourse.bacc as bacc
    import concourse.bass as bass
    import concourse.mybir as mybir
    import concourse.tile as tile

    f32 = mybir.dt.float32
    bf16 = mybir.dt.bfloat16
    i16 = mybir.dt.int16
    i32 = mybir.dt.int32
    AL = mybir.AluOpType

    S = cfg.slots_total
    SC = S // 128
    NBLK = cfg.nblk
    NW = cfg.n_win
    SEGC = cfg.seg_chunks
    BPQ = cfg.blocks_per_q
    QR = cfg.qrows
    gbase = [0]
    for gs in cfg.group_sizes:
        gbase.append(gbase[-1] + gs)

    nc = bacc.Bacc("TRN2", debug=False, num_devices=cfg.n_cores,
                   num_swdge_queues=4)
    P = nc.declare_dram_parameter

    gidx16 = P("gidx16", [128, S // 16], i16, isOutput=False)
    nrm_sm = P("nrm_sm", [128, SC], f32, isOutput=False)
    oh_hbm = P("oh_hbm", [128, SC, 128], bf16, isOutput=False)
    msg0_hbm = P("msg0_hbm", [128, SC, G], bf16, isOutput=False)
    selfw = P("selfw", [128, NBLK], f32, isOutput=False)
    batchloc = P("batchloc", [128, NBLK], bf16, isOutput=False)
    Xp = P("Xp", [cfg.core_rows, F], f32, isOutput=False)
    if use_virtual:
        vmaps = P("vmaps", [128, NW], i32, isOutput=False)
    inv_cnt = P("inv_cnt", [G, 1], f32, isOutput=False)
    W0T = P("W0T", [F, F], f32, isOutput=False)
    W1T = P("W1T", [F, F], f32, isOutput=False)
    W2 = P("W2", [F, F], f32, isOutput=False)
    out_ext = P("out", [G, F], f32, isOutput=True)

    # per-quarter shard (AG input / final u3) and window-assembled u (AG out)
    shard_q = [nc.dram_tensor(f"shard_q{q}", [QR, G], f32) for q in range(NW)]
    uq = [[nc.dram_tensor(f"u{p}_q{q}", [cfg.win, G], f32)
           for q in range(NW)] for p in range(2)]
    arin = nc.dram_tensor("arin", [G, F], f32)
    arout = nc.dram_tensor("arout", [G, F], f32)

    iota64_c = nc.inline_tensor(
        np.tile(np.arange(G, dtype=np.float32).astype(BF16), (128, 1)), "iota64")
    ident_c = nc.inline_tensor(np.eye(128, dtype=np.float32), "ident")

    qn = [0]

    def next_q():
        qn[0] = (qn[0] + 1) % 4
        return qn[0]

    with tile.TileContext(nc) as tc:
        with contextlib.ExitStack() as ctx:
            perm_pool = ctx.enter_context(tc.tile_pool(name="perm", bufs=1))
            acc = perm_pool.tile([128, NBLK, G], f32, tag="acc")
            selfw_sb = perm_pool.tile([128, NBLK], f32, tag="selfw")
            blc_sb = perm_pool.tile([128, NBLK], bf16, tag="blc")
            io64 = perm_pool.tile([128, G], bf16, tag="io64")
            ident_sb = perm_pool.tile([128, 128], f32, tag="ident")
            w_sb = perm_pool.tile([128, 3 * F], f32, tag="wsb")

            nc.sync.dma_start(out=selfw_sb[:], in_=selfw[:])
            nc.sync.dma_start(out=blc_sb[:], in_=batchloc[:])
            nc.sync.dma_start(out=io64[:], in_=iota64_c[:])
            nc.sync.dma_start(out=ident_sb[:], in_=ident_c[:])
            nc.sync.dma_start(out=w_sb[:, 0:F], in_=W0T[:])
            nc.sync.dma_start(out=w_sb[:, F:2 * F], in_=W1T[:])
            nc.sync.dma_start(out=w_sb[:, 2 * F:3 * F], in_=W2[:])

            raw_pool = ctx.enter_context(tc.tile_pool(name="raw", bufs=6))
            msg_pool = ctx.enter_context(tc.tile_pool(name="msg", bufs=6))
            oh_pool = ctx.enter_context(tc.tile_pool(name="oh", bufs=6))
            up_pool = ctx.enter_context(tc.tile_pool(name="up", bufs=2))
            ps_pool = ctx.enter_context(tc.tile_pool(name="ps", bufs=2, space="PSUM"))
            ep_pool = ctx.enter_context(tc.tile_pool(name="ep", bufs=2, space="PSUM"))
            fin_pool = ctx.enter_context(tc.tile_pool(name="fin", bufs=2))
            xp_pool = ctx.enter_context(tc.tile_pool(name="xp", bufs=2))
            if use_virtual:
                vps_pool = ctx.enter_context(
                    tc.tile_pool(name="vps", bufs=2, space="PSUM"))
                vmap_sb = perm_pool.tile([128, NW], i32, tag="vmap")
                nc.sync.dma_start(out=vmap_sb[:], in_=vmaps[:])

            # static per-slot aux data, resident in SBUF for all passes
            gidx_sb = perm_pool.tile([128, S // 16], i16, tag="gidx")
            nrm_sb = perm_pool.tile([128, SC], f32, tag="nrmsb")
            nc.sync.dma_start(out=gidx_sb[:], in_=gidx16[:])
            nc.sync.dma_start(out=nrm_sb[:], in_=nrm_sm[:])

            # epilogue weight chain early (overlaps pass 0):
            # W12 = W1 @ W2 ; W012 = W0 @ W12
            wps = ep_pool.tile([128, F], f32, tag="ep")
            w12 = fin_pool.tile([128, F], f32, tag="w12")
            nc.tensor.matmul(wps[:], lhsT=w_sb[:, F:2 * F],
                             rhs=w_sb[:, 2 * F:3 * F], start=True, stop=True)
            nc.vector.tensor_copy(out=w12[:], in_=wps[:])
            wps2 = ep_pool.tile([128, F], f32, tag="ep")
            w012 = perm_pool.tile([128, F], f32, tag="w012")
            nc.tensor.matmul(wps2[:], lhsT=w_sb[:, 0:F], rhs=w12[:],
                             start=True, stop=True)
            nc.vector.tensor_copy(out=w012[:], in_=wps2[:])

            # [QR, G] viewed as [128p, BPQ, G]
            shard_pbf = [shard_q[q][:].rearrange("(b p) f -> p b f", p=128)
                         for q in range(NW)]
            max_gs = max(cfg.group_sizes)

            def build_msgs(pk, g, w, C, soff):
                """Returns (msgt, oht) for the (g, w) slot region."""
                coff = soff // 128
                oht = oh_pool.tile([128, C, 128], bf16, tag="oh")
                nc.sync.dma_start(out=oht[:, :, :],
                                  in_=oh_hbm[:, coff:coff + C, :])
                if pk == 0:
                    msgt = msg_pool.tile([128, C, G], bf16, tag="msg")
                    nc.sync.dma_start(out=msgt[:, :, :],
                                      in_=msg0_hbm[:, coff:coff + C, :])
                else:
                    src = uq[(pk + 1) % 2][w]
                    rawt = raw_pool.tile([128, C, G], f32, tag="raw")
                    for sub in range(0, C, cfg.csub):
                        cs = min(cfg.csub, C - sub)
                        nc.gpsimd.dma_gather(
                            rawt[:, sub:sub + cs, :],
                            src[:, :],
                            gidx_sb[:, soff // 16 + sub * 8:
                                    soff // 16 + (sub + cs) * 8],
                            cs * 128, cs * 128, G,
                            single_packet=False,
                            queue_num=next_q())
                    msgt = msg_pool.tile([128, C, G], bf16, tag="msg")
                    nc.vector.tensor_tensor(
                        out=msgt[:, :, :], in0=rawt[:, :, :],
                        in1=nrm_sb[:, coff:coff + C].to_broadcast([128, C, G]),
                        op=AL.mult)
                return msgt, oht

            def do_pass(pk):
                # virtual (spill) group first so vsb is ready per quarter
                if use_virtual:
                    vg = cfg.n_groups - 1
                    VC = cfg.vchunks
                    vsb = fin_pool.tile([128, G], f32, tag="vsb")
                    for w in range(NW):
                        soff = cfg.gw_slot_off[(vg, w)]
                        msgt, oht = build_msgs(pk, vg, w, VC, soff)
                        vps = vps_pool.tile([128, G], f32, tag="vps")
                        for ci in range(VC):
                            nc.tensor.matmul(
                                vps[:, :], lhsT=oht[:, ci, :],
                                rhs=msgt[:, ci, :],
                                start=(ci == 0), stop=(ci == VC - 1))
                        if w == 0:
                            nc.vector.tensor_copy(out=vsb[:], in_=vps[:])
                        else:
                            nc.vector.tensor_tensor(
                                out=vsb[:], in0=vsb[:], in1=vps[:], op=AL.add)

                for qo in range(NW):
                    for g in (2 * qo, 2 * qo + 1):
                        gs = cfg.group_sizes[g]
                        C = cfg.gw_chunks[g]
                        seg = slice(gbase[g], gbase[g] + gs)
                        # self-loop init of acc for this group
                        upt = up_pool.tile([128, gs, G], f32, tag="up")
                        if pk == 0:
                            nc.vector.tensor_tensor(
                                out=upt[:, :, :],
                                in0=blc_sb[:, seg].to_broadcast([128, gs, G]),
                                in1=_midbcast(io64[:], gs),
                                op=AL.is_equal)
                        else:
                            lo = gbase[g] - qo * BPQ
                            nc.sync.dma_start(
                                out=upt[:, :, :],
                                in_=shard_pbf[qo][:, lo:lo + gs, :])
                        nc.vector.tensor_tensor(
                            out=acc[:, seg, :], in0=upt[:, :, :],
                            in1=selfw_sb[:, seg].to_broadcast([128, gs, G]),
                            op=AL.mult)
                        # all 4 windows' msgs, then one contiguous
                        # accumulation chain per PSUM region
                        mos = [build_msgs(pk, g, w, C, cfg.gw_slot_off[(g, w)])
                               for w in range(NW)]
                        ps = ps_pool.tile([128, max_gs * G], f32, tag="ps")
                        for j in range(gs):
                            for w in range(NW):
                                msgt, oht = mos[w]
                                for s in range(SEGC):
                                    ci = j * SEGC + s
                                    nc.tensor.matmul(
                                        ps[:, j * G:(j + 1) * G],
                                        lhsT=oht[:, ci, :], rhs=msgt[:, ci, :],
                                        start=(w == 0 and s == 0),
                                        stop=(w == NW - 1 and s == SEGC - 1))
                        nc.vector.tensor_tensor(
                            out=acc[:, seg, :], in0=acc[:, seg, :],
                            in1=ps[:, :gs * G].rearrange(
                                "p (b f) -> p b f", f=G),
                            op=AL.add)
                    # quarter qo finalized: write shard, add spills, AllGather
                    nc.sync.dma_start(
                        out=shard_pbf[qo],
                        in_=acc[:, qo * BPQ:(qo + 1) * BPQ, :])
                    if use_virtual:
                        nc.gpsimd.indirect_dma_start(
                            out=shard_q[qo][:, :],
                            out_offset=bass.IndirectOffsetOnAxis(
                                ap=vmap_sb[:, qo:qo + 1], axis=0),
                            in_=vsb[:, :], in_offset=None,
                            bounds_check=QR - 1, oob_is_err=False,
                            compute_op=AL.add)
                    if pk < LAYERS - 1:
                        nc.gpsimd.collective_compute(
                            "AllGather", AL.bypass,
                            replica_groups=[list(range(cfg.n_cores))],
                            ins=[shard_q[qo][:]], outs=[uq[pk % 2][qo][:]])

            for pk in range(LAYERS):
                do_pass(pk)

            # final dense matmul: out_part[g, f] = sum_n u3[n, g] * Xp[n, f]
            fps = ep_pool.tile([G, F], f32, tag="ep")
            xp_pbf = Xp[:].rearrange("(b p) f -> p b f", p=128)
            for qo in range(NW):
                u3 = fin_pool.tile([128, BPQ, G], f32, tag="u3")
                nc.sync.dma_start(out=u3[:, :, :], in_=shard_pbf[qo])
                for half in range(2):
                    hb = (BPQ + 1) // 2 if half == 0 else BPQ - (BPQ + 1) // 2
                    hoff = 0 if half == 0 else (BPQ + 1) // 2
                    xpt = xp_pool.tile([128, (BPQ + 1) // 2, F], f32, tag="xp")
                    nc.sync.dma_start(
                        out=xpt[:, :hb, :],
                        in_=xp_pbf[:, qo * BPQ + hoff:qo * BPQ + hoff + hb, :])
                    for b in range(hb):
                        gb = qo * BPQ + hoff + b
                        nc.tensor.matmul(fps[:], lhsT=u3[:, hoff + b, :],
                                         rhs=xpt[:, b, :],
                                         start=(gb == 0), stop=(gb == NBLK - 1))
            outp = fin_pool.tile([G, F], f32, tag="outp")
            nc.vector.tensor_copy(out=outp[:], in_=fps[:])
            nc.sync.dma_start(out=arin[:], in_=outp[:])
            nc.gpsimd.collective_compute(
                "AllReduce", AL.add,
                replica_groups=[list(range(cfg.n_cores))],
                ins=[arin[:]], outs=[arout[:]])
            ar_sb = fin_pool.tile([G, F], f32, tag="arsb")
            nc.sync.dma_start(out=ar_sb[:], in_=arout[:])

            # epilogue: res^T ; out = (res^T W012) / counts
            tps = ep_pool.tile([128, G], f32, tag="ep")
            nc.tensor.transpose(out=tps[:], in_=ar_sb[:, :],
                                identity=ident_sb[:G, :G])
            resT = fin_pool.tile([128, G], f32, tag="resT")
            nc.vector.tensor_copy(out=resT[:], in_=tps[:])
            ops = ep_pool.tile([G, F], f32, tag="ep")
            nc.tensor.matmul(ops[:], lhsT=resT[:], rhs=w012[:], start=True,
                             stop=True)
            icnt = fin_pool.tile([G, 1], f32, tag="icnt")
            nc.sync.dma_start(out=icnt[:], in_=inv_cnt[:])
            fin = fin_pool.tile([G, F], f32, tag="finout")
            nc.vector.tensor_scalar_mul(fin[:], ops[:], icnt[:])
            nc.sync.dma_start(out=out_ext[:], in_=fin[:])

    nc.compile()
    return nc


def make_in_maps(cfg, aux):
    in_maps = []
    for c in range(cfg.n_cores):
        in_maps.append({
            "gidx16": np.ascontiguousarray(aux["gidx16"][c]),
            "nrm_sm": np.ascontiguousarray(aux["nrm_sm"][c]),
            "oh_hbm": np.ascontiguousarray(aux["oh_hbm"][c]),
            "msg0_hbm": np.ascontiguousarray(aux["msg0_hbm"][c]),
            "selfw": np.ascontiguousarray(aux["selfw"][c]),
            "batchloc": np.ascontiguousarray(aux["batchloc"][c]),
            "Xp": np.ascontiguousarray(aux["Xp"][c]),
            "inv_cnt": aux["inv_cnt"],
            "W0T": aux["W0T"], "W1T": aux["W1T"], "W2": aux["W2"],
        })
        if aux["spills"] > 0:
            in_maps[-1]["vmaps"] = np.ascontiguousarray(aux["vmaps"][c])
    return in_maps


_PROGRAM_CACHE = {}


def kernel(**inputs):
    from concourse.bass_utils import run_bass_kernel_spmd

    cfg = FULL_CFG
    x = np.asarray(inputs["x"], dtype=np.float32)
    edge_index = np.asarray(inputs["edge_index"])
    edge_attr = np.asarray(inputs["edge_attr"], dtype=np.float32)
    batch = np.asarray(inputs["batch"])
    Ws = np.asarray(inputs["Ws"], dtype=np.float32)
    bs = np.asarray(inputs["bs"], dtype=np.float32)
    assert not np.any(bs), "nonzero biases not supported by this kernel build"

    aux = host_prep(cfg, x, edge_index, edge_attr, batch, Ws, bs)
    use_virtual = aux["spills"] > 0
    key = ("full", cfg.slots_total, use_virtual)
    if key not in _PROGRAM_CACHE:
        _PROGRAM_CACHE[key] = build_program(cfg, use_virtual=use_virtual)
    nc = _PROGRAM_CACHE[key]
    in_maps = make_in_maps(cfg, aux)
    res = run_bass_kernel_spmd(nc, in_maps, core_ids=list(range(cfg.n_cores)))
    return np.asarray(res.results[0]["out"], dtype=np.float32)


# revision 12
# speedup vs baseline: 1.7927x; 1.1060x over previous
"""Trainium2 Bass kernel for CellGraphSignatureGNN (GCN message passing).

Math: the network is affine per layer: x_{l+1} = A @ x_l @ W_l + 1 b_l^T,
with A = D^-1/2 (Adj + 2I) D^-1/2 (weighted adjacency + improved self loops),
followed by a per-graph mean pool P (and bs == 0 in this problem).  Since A
acts on nodes and W on features:

    out = P A^3 X (W0 W1 W2) / counts

We evaluate left-to-right: u1^T = A^T P^T, u2^T = A^T u1^T, u3^T = A^T u2^T
(64-wide node vectors), then one dense matmul (u3 X) sharded over nodes, a
tiny AllReduce, and the 128x128 weight chain on-device.

Distribution: nodes are re-labeled and bin-packed into 128-node blocks (100
blocks/core x 8 cores) balanced by scatter-degree so every core runs an
IDENTICAL (SPMD) program; per-core data (gather indices, one-hot offsets,
norms) differ.  Edge (r, c) contributes norm_e * u[c] into u_new[r]:
  - gather u[c] rows (256B fp32) from the window-replicated u in HBM via
    dma_gather (int16 indices), round-robined over all 4 SWDGE queues so all
    8 GpSimd Q7 cores generate DMA descriptors in parallel,
  - scale by norm_e on DVE (bf16 out), build a destination one-hot on DVE,
  - scatter = one-hot matmul accumulated in PSUM per 128-dest block,
  - the per-layer u re-replication is FOUR quarter AllGathers (Shared outputs)
    that pipeline with compute: quarter q's AG is issued as soon as its 25
    blocks are final, and the next pass's window-w work only waits for AG w.
Self-loops are applied as local elementwise ops; rare (block,window) segment
overflow edges go through a "virtual block" + per-quarter indirect
scatter-add with CCE.
"""

import numpy as np
import ml_dtypes

BF16 = ml_dtypes.bfloat16

G = 64        # graphs
F = 128       # feature width
LAYERS = 3
PAD_SENT = 30000.0  # destoff/bg sentinel for padded slots (one-hot -> 0)


# --------------------------------------------------------------------------
# configuration
# --------------------------------------------------------------------------
class Cfg:
    def __init__(self, n_nodes, n_edges, n_cores=8, nblk=100, group_sizes=None,
                 seg_chunks=2, n_win=4, vchunks=2, vcap=128, csub=8):
        self.n_nodes = n_nodes
        self.n_edges = n_edges
        self.n_cores = n_cores
        self.nblk = nblk                      # real blocks per core
        self.group_sizes = group_sizes or [13, 12] * 4
        assert sum(self.group_sizes) == nblk
        self.seg_chunks = seg_chunks          # 128-slot chunks per (block, window)
        self.n_win = n_win
        self.vchunks = vchunks                # chunks per window for the virtual blk
        self.vcap = vcap                      # distinct spill destinations
        self.csub = csub                      # chunks per dma_gather call
        self.core_rows = nblk * 128           # permuted rows per core
        self.pn = n_cores * self.core_rows    # total permuted rows
        assert self.core_rows % n_win == 0
        self.qrows = self.core_rows // n_win  # rows per (core, quarter)
        self.win = n_cores * self.qrows       # rows per assembled window
        assert self.win <= 32768
        assert nblk % n_win == 0
        self.blocks_per_q = nblk // n_win
        # groups must tile quarters exactly: groups [2q], [2q+1] cover quarter q
        assert len(self.group_sizes) == 2 * n_win
        for q in range(n_win):
            assert (self.group_sizes[2 * q] + self.group_sizes[2 * q + 1]
                    == self.blocks_per_q)
        # slot layout: for g in groups + [vgroup]: for w in windows: contig slots
        self.gw_chunks = [gs * seg_chunks for gs in self.group_sizes] + [vchunks]
        self.n_groups = len(self.gw_chunks)   # includes virtual group
        self.slots_total = sum(self.gw_chunks) * 128 * n_win
        self.gw_slot_off = {}
        off = 0
        for g, c in enumerate(self.gw_chunks):
            for w in range(n_win):
                self.gw_slot_off[(g, w)] = off
                off += c * 128
        assert off == self.slots_total


FULL_CFG = Cfg(100000, 640000)


# --------------------------------------------------------------------------
# host-side graph preprocessing (indices, norms, schedules)
# --------------------------------------------------------------------------
def host_prep(cfg, x, edge_index, edge_attr, batch, Ws, bs):
    N, E = cfg.n_nodes, cfg.n_edges
    row = np.asarray(edge_index[0], dtype=np.int64)
    col = np.asarray(edge_index[1], dtype=np.int64)
    w = np.asarray(edge_attr, dtype=np.float32).reshape(-1)
    batch = np.asarray(batch, dtype=np.int64)

    deg = np.zeros(N, dtype=np.float64)
    np.add.at(deg, col, w.astype(np.float64))
    deg += 2.0
    dinv = (1.0 / np.sqrt(deg)).astype(np.float32)
    norm = dinv[row] * w * dinv[col]
    selfnorm = 2.0 * dinv * dinv
    cnt = np.bincount(batch, minlength=G).astype(np.float32)

    # ---- bin-pack nodes into blocks by scatter degree (edges with row == n)
    sdeg = np.bincount(row, minlength=N)
    nbins = cfg.n_cores * cfg.nblk
    order = np.argsort(-sdeg, kind="stable")
    binsum = np.zeros(nbins, dtype=np.int64)
    binfill = np.zeros(nbins, dtype=np.int32)
    import heapq
    heap = [(0, b) for b in range(nbins)]
    heapq.heapify(heap)
    node_bin = np.empty(N, dtype=np.int32)
    node_pos = np.empty(N, dtype=np.int32)
    for n in order:
        while True:
            s, b = heapq.heappop(heap)
            if binfill[b] < 128:
                break
        node_bin[n] = b
        node_pos[n] = binfill[b]
        binfill[b] += 1
        binsum[b] += sdeg[n]
        if binfill[b] < 128:
            heapq.heappush(heap, (int(binsum[b]), b))
    # snake-assign bins to cores by load
    border = np.argsort(-binsum, kind="stable")
    bin_core = np.empty(nbins, dtype=np.int32)
    bin_blk = np.empty(nbins, dtype=np.int32)
    percore = [[] for _ in range(cfg.n_cores)]
    for i, b in enumerate(border):
        r = i // cfg.n_cores
        k = i % cfg.n_cores
        c = k if (r % 2 == 0) else cfg.n_cores - 1 - k
        bin_core[b] = c
        bin_blk[b] = len(percore[c])
        percore[c].append(b)
    assert all(len(p) == cfg.nblk for p in percore)

    # core-local row of a node; window = quarter of the local row
    local_row = bin_blk[node_bin].astype(np.int64) * 128 + node_pos
    node_core = bin_core[node_bin].astype(np.int64)
    node_q = local_row // cfg.qrows
    node_widx = node_core * cfg.qrows + (local_row % cfg.qrows)

    # ---- per-core edge schedules
    e_core = bin_core[node_bin[row]]
    e_blk = bin_blk[node_bin[row]]
    e_doff = node_pos[row]                 # dest offset within block
    e_win = node_q[col].astype(np.int32)   # source window (quarter)
    e_gidx = node_widx[col].astype(np.int32)  # gather idx within window

    SEG = cfg.seg_chunks * 128
    S = cfg.slots_total
    n_cores = cfg.n_cores

    gidx = np.zeros((n_cores, S), dtype=np.int32)
    doff = np.full((n_cores, S), PAD_SENT, dtype=np.float32)
    nrm = np.zeros((n_cores, S), dtype=np.float32)
    bg = np.full((n_cores, S), PAD_SENT, dtype=np.float32)

    # block -> (group, block-in-group) mapping
    blk_group = []
    blk_ing = []
    for g, gs in enumerate(cfg.group_sizes):
        for j in range(gs):
            blk_group.append(g)
            blk_ing.append(j)
    blk_group = np.array(blk_group)
    blk_ing = np.array(blk_ing)

    # per-quarter rebased virtual spill maps (1<<30 = inactive / out of range)
    vmaps = np.full((n_cores, 128, cfg.n_win), 1 << 30, dtype=np.int32)
    spill_warn = 0

    for c in range(n_cores):
        em = e_core == c
        eb = e_blk[em]
        ew = e_win[em]
        eg = e_gidx[em]
        ed = e_doff[em]
        en = norm[em]
        ebg = batch[col[em]].astype(np.float32)
        # order edges by (block, window)
        key = eb * cfg.n_win + ew
        o = np.argsort(key, kind="stable")
        eb, ew, eg, ed, en, ebg = eb[o], ew[o], eg[o], ed[o], en[o], ebg[o]
        spill_list = []
        kk = eb * cfg.n_win + ew
        bounds = np.searchsorted(kk, np.arange(cfg.nblk * cfg.n_win + 1))
        vused = {}
        for b in range(cfg.nblk):
            g = blk_group[b]
            j = blk_ing[b]
            for wi in range(cfg.n_win):
                lo, hi = bounds[b * cfg.n_win + wi], bounds[b * cfg.n_win + wi + 1]
                nseg = hi - lo
                take = min(nseg, SEG)
                base = (cfg.gw_slot_off[(g, wi)] + j * SEG)
                sl = slice(base, base + take)
                gidx[c, sl] = eg[lo:lo + take]
                doff[c, sl] = ed[lo:lo + take]
                nrm[c, sl] = en[lo:lo + take]
                bg[c, sl] = ebg[lo:lo + take]
                for t in range(lo + take, hi):
                    spill_list.append((b, ew[t], eg[t], ed[t], en[t], ebg[t]))
        # spills into virtual group
        vg = cfg.n_groups - 1
        vfill = np.zeros(cfg.n_win, dtype=np.int32)
        for (b, wi, gg, dd, nn, bb) in spill_list:
            key2 = (b, dd)
            if key2 not in vused:
                assert len(vused) < cfg.vcap, "virtual dest capacity exceeded"
                v = len(vused)
                vused[key2] = v
                r = b * 128 + dd
                q = r // cfg.qrows
                vmaps[c, v, q] = r - q * cfg.qrows
            v = vused[key2]
            assert vfill[wi] < cfg.vchunks * 128, "virtual slot capacity exceeded"
            base = cfg.gw_slot_off[(vg, wi)] + vfill[wi]
            gidx[c, base] = gg
            doff[c, base] = v
            nrm[c, base] = nn
            bg[c, base] = bb
            vfill[wi] += 1
        spill_warn += len(spill_list)

    # ---- pack aux arrays
    ncol16 = S // 16
    gidx16 = np.zeros((n_cores, 128, ncol16), dtype=np.int16)
    s_idx = np.arange(S)
    for c in range(n_cores):
        lay = np.zeros((16, ncol16), dtype=np.int16)
        lay[s_idx % 16, s_idx // 16] = (gidx[c] // 2).astype(np.int16)
        gidx16[c] = np.tile(lay, (8, 1))
    ncol128 = S // 128

    def slotmajor(a, dt):
        out = np.zeros((n_cores, 128, ncol128), dtype=dt)
        for c in range(n_cores):
            out[c][s_idx % 128, s_idx // 128] = a[c]
        return out

    nrm_sm = slotmajor(nrm, np.float32)
    doff_sm = slotmajor(doff, BF16)
    bg_sm = slotmajor(bg, BF16)
    # pair-gather: index = source-pair row; a/b select the even/odd half
    odd = (gidx % 2).astype(np.float32)
    a_sm = slotmajor(nrm * (1.0 - odd), BF16)
    b_sm = slotmajor(nrm * odd, BF16)

    # host-prebuilt destination one-hots (all passes) and pass-0 messages
    SC = S // 128
    p_i = s_idx % 128
    c_i = s_idx // 128
    oh_hbm = np.zeros((n_cores, 128, SC, 128), dtype=BF16)
    msg0_hbm = np.zeros((n_cores, 128, SC, G), dtype=BF16)
    for c in range(n_cores):
        dv = doff[c].astype(np.int64)
        m = doff[c] != PAD_SENT
        oh_hbm[c][p_i[m], c_i[m], dv[m]] = 1
        bv = bg[c].astype(np.int64)
        mb = bg[c] != PAD_SENT
        msg0_hbm[c][p_i[mb], c_i[mb], bv[mb]] = nrm[c][mb].astype(BF16)

    # ---- per-core node-level aux
    selfw = np.zeros((n_cores, 128, cfg.nblk), dtype=np.float32)
    batchloc = np.full((n_cores, 128, cfg.nblk), PAD_SENT, dtype=BF16)
    Xp = np.zeros((n_cores, cfg.core_rows, F), dtype=np.float32)
    nodes = np.arange(N)
    pc = bin_core[node_bin]
    pb = bin_blk[node_bin]
    pp = node_pos
    for c in range(n_cores):
        m = pc == c
        selfw[c][pp[m], pb[m]] = selfnorm[nodes[m]]
        batchloc[c][pp[m], pb[m]] = batch[nodes[m]].astype(np.float32)
        Xp[c][pb[m] * 128 + pp[m]] = np.asarray(x, dtype=np.float32)[nodes[m]]

    inv_cnt = (1.0 / np.maximum(cnt, 1.0)).astype(np.float32).reshape(G, 1)
    Ws = np.asarray(Ws, dtype=np.float32)
    bs = np.asarray(bs, dtype=np.float32)

    aux = dict(
        gidx16=gidx16, nrm_sm=nrm_sm, doff_sm=doff_sm, bg_sm=bg_sm,
        a_sm=a_sm, b_sm=b_sm, Xpb=Xp.astype(BF16),
        oh_hbm=oh_hbm, msg0_hbm=msg0_hbm,
        selfw=selfw, batchloc=batchloc, Xp=Xp, vmaps=vmaps,
        inv_cnt=inv_cnt,
        W0T=np.ascontiguousarray(Ws[0].T), W1T=np.ascontiguousarray(Ws[1].T),
        W2=np.ascontiguousarray(Ws[2]), bs=bs,
        spills=spill_warn,
    )
    return aux


def _midbcast(ap, count):
    """Insert a step-0 middle axis: [P, X] -> [P, (0,count), X]."""
    import concourse.bass as bass
    assert len(ap.ap) == 2
    return bass.AP(ap.tensor, ap.offset, [ap.ap[0], [0, count], ap.ap[1]])


def build_program(cfg, use_virtual=True):
    import contextlib
    import concourse.bacc as bacc
    import concourse.bass as bass
    import concourse.mybir as mybir
    import concourse.tile as tile

    f32 = mybir.dt.float32
    bf16 = mybir.dt.bfloat16
    i16 = mybir.dt.int16
    i32 = mybir.dt.int32
    AL = mybir.AluOpType

    S = cfg.slots_total
    SC = S // 128
    NBLK = cfg.nblk
    NW = cfg.n_win
    SEGC = cfg.seg_chunks
    BPQ = cfg.blocks_per_q
    QR = cfg.qrows
    gbase = [0]
    for gs in cfg.group_sizes:
        gbase.append(gbase[-1] + gs)

    nc = bacc.Bacc("TRN2", debug=False, num_devices=cfg.n_cores,
                   num_swdge_queues=4)
    P = nc.declare_dram_parameter

    gidx16 = P("gidx16", [128, S // 16], i16, isOutput=False)
    a_sm = P("a_sm", [128, SC], bf16, isOutput=False)
    b_sm = P("b_sm", [128, SC], bf16, isOutput=False)
    oh_hbm = P("oh_hbm", [128, SC, 128], bf16, isOutput=False)
    msg0_hbm = P("msg0_hbm", [128, SC, G], bf16, isOutput=False)
    selfw = P("selfw", [128, NBLK], f32, isOutput=False)
    batchloc = P("batchloc", [128, NBLK], bf16, isOutput=False)
    Xp = P("Xpb", [cfg.core_rows, F], bf16, isOutput=False)
    if use_virtual:
        vmaps = P("vmaps", [128, NW], i32, isOutput=False)
    inv_cnt = P("inv_cnt", [G, 1], f32, isOutput=False)
    W0T = P("W0T", [F, F], f32, isOutput=False)
    W1T = P("W1T", [F, F], f32, isOutput=False)
    W2 = P("W2", [F, F], f32, isOutput=False)
    out_ext = P("out", [G, F], f32, isOutput=True)

    # per-quarter shard (AG input / final u3) and window-assembled u (AG out)
    shard_q = [nc.dram_tensor(f"shard_q{q}", [QR, G], bf16) for q in range(NW)]
    uq = [[nc.dram_tensor(f"u{p}_q{q}", [cfg.win, G], bf16)
           for q in range(NW)] for p in range(2)]
    arin = nc.dram_tensor("arin", [G, F], f32)
    arout = nc.dram_tensor("arout", [G, F], f32)

    iota64_c = nc.inline_tensor(
        np.tile(np.arange(G, dtype=np.float32).astype(BF16), (128, 1)), "iota64")
    ident_c = nc.inline_tensor(np.eye(128, dtype=np.float32), "ident")

    qn = [0]

    def next_q():
        qn[0] = (qn[0] + 1) % 4
        return qn[0]

    with tile.TileContext(nc) as tc:
        with contextlib.ExitStack() as ctx:
            perm_pool = ctx.enter_context(tc.tile_pool(name="perm", bufs=1))
            acc = perm_pool.tile([128, NBLK, G], bf16, tag="acc")
            selfw_sb = perm_pool.tile([128, NBLK], f32, tag="selfw")
            blc_sb = perm_pool.tile([128, NBLK], bf16, tag="blc")
            io64 = perm_pool.tile([128, G], bf16, tag="io64")
            ident_sb = perm_pool.tile([128, 128], f32, tag="ident")
            w_sb = perm_pool.tile([128, 3 * F], f32, tag="wsb")

            nc.sync.dma_start(out=selfw_sb[:], in_=selfw[:])
            nc.sync.dma_start(out=blc_sb[:], in_=batchloc[:])
            nc.sync.dma_start(out=io64[:], in_=iota64_c[:])
            nc.sync.dma_start(out=ident_sb[:], in_=ident_c[:])
            nc.sync.dma_start(out=w_sb[:, 0:F], in_=W0T[:])
            nc.sync.dma_start(out=w_sb[:, F:2 * F], in_=W1T[:])
            nc.sync.dma_start(out=w_sb[:, 2 * F:3 * F], in_=W2[:])

            raw_pool = ctx.enter_context(tc.tile_pool(name="raw", bufs=6))
            msg_pool = ctx.enter_context(tc.tile_pool(name="msg", bufs=6))
            oh_pool = ctx.enter_context(tc.tile_pool(name="oh", bufs=6))
            up_pool = ctx.enter_context(tc.tile_pool(name="up", bufs=2))
            ps_pool = ctx.enter_context(tc.tile_pool(name="ps", bufs=2, space="PSUM"))
            ep_pool = ctx.enter_context(tc.tile_pool(name="ep", bufs=2, space="PSUM"))
            fin_pool = ctx.enter_context(tc.tile_pool(name="fin", bufs=2))
            xp_pool = ctx.enter_context(tc.tile_pool(name="xp", bufs=2))
            if use_virtual:
                vps_pool = ctx.enter_context(
                    tc.tile_pool(name="vps", bufs=2, space="PSUM"))
                vmap_sb = perm_pool.tile([128, NW], i32, tag="vmap")
                nc.sync.dma_start(out=vmap_sb[:], in_=vmaps[:])

            # static per-slot aux data, resident in SBUF for all passes
            gidx_sb = perm_pool.tile([128, S // 16], i16, tag="gidx")
            a_sb = perm_pool.tile([128, SC], bf16, tag="asb")
            b_sb = perm_pool.tile([128, SC], bf16, tag="bsb")
            nc.sync.dma_start(out=gidx_sb[:], in_=gidx16[:])
            nc.sync.dma_start(out=a_sb[:], in_=a_sm[:])
            nc.sync.dma_start(out=b_sb[:], in_=b_sm[:])

            # epilogue weight chain early (overlaps pass 0):
            # W12 = W1 @ W2 ; W012 = W0 @ W12
            wps = ep_pool.tile([128, F], f32, tag="ep")
            w12 = fin_pool.tile([128, F], f32, tag="w12")
            nc.tensor.matmul(wps[:], lhsT=w_sb[:, F:2 * F],
                             rhs=w_sb[:, 2 * F:3 * F], start=True, stop=True)
            nc.vector.tensor_copy(out=w12[:], in_=wps[:])
            wps2 = ep_pool.tile([128, F], f32, tag="ep")
            w012 = perm_pool.tile([128, F], f32, tag="w012")
            nc.tensor.matmul(wps2[:], lhsT=w_sb[:, 0:F], rhs=w12[:],
                             start=True, stop=True)
            nc.vector.tensor_copy(out=w012[:], in_=wps2[:])

            # [QR, G] viewed as [128p, BPQ, G]
            shard_pbf = [shard_q[q][:].rearrange("(b p) f -> p b f", p=128)
                         for q in range(NW)]
            max_gs = max(cfg.group_sizes)

            def build_msgs(pk, g, w, C, soff):
                """Returns (msgt, oht) for the (g, w) slot region."""
                coff = soff // 128
                oht = oh_pool.tile([128, C, 128], bf16, tag="oh")
                nc.scalar.dma_start(out=oht[:, :, :],
                                    in_=oh_hbm[:, coff:coff + C, :])
                if pk == 0:
                    msgt = msg_pool.tile([128, C, G], bf16, tag="msg")
                    nc.sync.dma_start(out=msgt[:, :, :],
                                      in_=msg0_hbm[:, coff:coff + C, :])
                else:
                    src = uq[(pk + 1) % 2][w][:].rearrange(
                        "(p two) f -> p (two f)", two=2)
                    rawt = raw_pool.tile([128, C, 2 * G], bf16, tag="raw")
                    for sub in range(0, C, cfg.csub):
                        cs = min(cfg.csub, C - sub)
                        nc.gpsimd.dma_gather(
                            rawt[:, sub:sub + cs, :],
                            src,
                            gidx_sb[:, soff // 16 + sub * 8:
                                    soff // 16 + (sub + cs) * 8],
                            cs * 128, cs * 128, 2 * G,
                            single_packet=False,
                            queue_num=next_q())
                    msgt = msg_pool.tile([128, C, G], bf16, tag="msg")
                    tmpt = msg_pool.tile([128, C, G], bf16, tag="msgtmp")
                    nc.vector.tensor_tensor(
                        out=msgt[:, :, :], in0=rawt[:, :, 0:G],
                        in1=a_sb[:, coff:coff + C].to_broadcast([128, C, G]),
                        op=AL.mult)
                    nc.vector.tensor_tensor(
                        out=tmpt[:, :, :], in0=rawt[:, :, G:2 * G],
                        in1=b_sb[:, coff:coff + C].to_broadcast([128, C, G]),
                        op=AL.mult)
                    nc.vector.tensor_tensor(
                        out=msgt[:, :, :], in0=msgt[:, :, :],
                        in1=tmpt[:, :, :], op=AL.add)
                return msgt, oht

            def do_pass(pk):
                # virtual (spill) group first so vsb is ready per quarter
                if use_virtual:
                    vg = cfg.n_groups - 1
                    VC = cfg.vchunks
                    vsb = fin_pool.tile([128, G], f32, tag="vsb")
                    for w in range(NW):
                        soff = cfg.gw_slot_off[(vg, w)]
                        msgt, oht = build_msgs(pk, vg, w, VC, soff)
                        vps = vps_pool.tile([128, G], f32, tag="vps")
                        for ci in range(VC):
                            nc.tensor.matmul(
                                vps[:, :], lhsT=oht[:, ci, :],
                                rhs=msgt[:, ci, :],
                                start=(ci == 0), stop=(ci == VC - 1))
                        if w == 0:
                            nc.vector.tensor_copy(out=vsb[:], in_=vps[:])
                        else:
                            nc.vector.tensor_tensor(
                                out=vsb[:], in0=vsb[:], in1=vps[:], op=AL.add)

                for qo in range(NW):
                    for g in (2 * qo, 2 * qo + 1):
                        gs = cfg.group_sizes[g]
                        C = cfg.gw_chunks[g]
                        seg = slice(gbase[g], gbase[g] + gs)
                        # self-loop init of acc for this group
                        upt = up_pool.tile([128, gs, G], bf16, tag="up")
                        if pk == 0:
                            nc.vector.tensor_tensor(
                                out=upt[:, :, :],
                                in0=blc_sb[:, seg].to_broadcast([128, gs, G]),
                                in1=_midbcast(io64[:], gs),
                                op=AL.is_equal)
                        else:
                            lo = gbase[g] - qo * BPQ
                            nc.sync.dma_start(
                                out=upt[:, :, :],
                                in_=shard_pbf[qo][:, lo:lo + gs, :])
                        nc.vector.tensor_tensor(
                            out=acc[:, seg, :], in0=upt[:, :, :],
                            in1=selfw_sb[:, seg].to_broadcast([128, gs, G]),
                            op=AL.mult)
                        # all 4 windows' msgs, then one contiguous
                        # accumulation chain per PSUM region
                        mos = [build_msgs(pk, g, w, C, cfg.gw_slot_off[(g, w)])
                               for w in range(NW)]
                        ps = ps_pool.tile([128, max_gs * G], f32, tag="ps")
                        for j in range(gs):
                            for w in range(NW):
                                msgt, oht = mos[w]
                                for s in range(SEGC):
                                    ci = j * SEGC + s
                                    nc.tensor.matmul(
                                        ps[:, j * G:(j + 1) * G],
                                        lhsT=oht[:, ci, :], rhs=msgt[:, ci, :],
                                        start=(w == 0 and s == 0),
                                        stop=(w == NW - 1 and s == SEGC - 1))
                        nc.vector.tensor_tensor(
                            out=acc[:, seg, :], in0=acc[:, seg, :],
                            in1=ps[:, :gs * G].rearrange(
                                "p (b f) -> p b f", f=G),
                            op=AL.add)
                    # quarter qo finalized: write shard, add spills, AllGather
                    nc.sync.dma_start(
                        out=shard_pbf[qo],
                        in_=acc[:, qo * BPQ:(qo + 1) * BPQ, :])
                    if use_virtual:
                        nc.gpsimd.indirect_dma_start(
                            out=shard_q[qo][:, :],
                            out_offset=bass.IndirectOffsetOnAxis(
                                ap=vmap_sb[:, qo:qo + 1], axis=0),
                            in_=vsb[:, :], in_offset=None,
                            bounds_check=QR - 1, oob_is_err=False,
                            compute_op=AL.add)
                    if pk < LAYERS - 1:
                        nc.gpsimd.collective_compute(
                            "AllGather", AL.bypass,
                            replica_groups=[list(range(cfg.n_cores))],
                            ins=[shard_q[qo][:]], outs=[uq[pk % 2][qo][:]])

            for pk in range(LAYERS):
                do_pass(pk)

            # final dense matmul: out_part[g, f] = sum_n u3[n, g] * Xp[n, f]
            fps = ep_pool.tile([G, F], f32, tag="ep")
            xp_pbf = Xp[:].rearrange("(b p) f -> p b f", p=128)
            for qo in range(NW):
                u3 = fin_pool.tile([128, BPQ, G], bf16, tag="u3")
                nc.sync.dma_start(out=u3[:, :, :], in_=shard_pbf[qo])
                xpt = xp_pool.tile([128, BPQ, F], bf16, tag="xp")
                nc.sync.dma_start(
                    out=xpt[:, :, :],
                    in_=xp_pbf[:, qo * BPQ:(qo + 1) * BPQ, :])
                for b in range(BPQ):
                    gb = qo * BPQ + b
                    nc.tensor.matmul(fps[:], lhsT=u3[:, b, :],
                                     rhs=xpt[:, b, :],
                                     start=(gb == 0), stop=(gb == NBLK - 1))
            outp = fin_pool.tile([G, F], f32, tag="outp")
            nc.vector.tensor_copy(out=outp[:], in_=fps[:])
            nc.sync.dma_start(out=arin[:], in_=outp[:])
            nc.gpsimd.collective_compute(
                "AllReduce", AL.add,
                replica_groups=[list(range(cfg.n_cores))],
                ins=[arin[:]], outs=[arout[:]])
            ar_sb = fin_pool.tile([G, F], f32, tag="arsb")
            nc.sync.dma_start(out=ar_sb[:], in_=arout[:])

            # epilogue: res^T ; out = (res^T W012) / counts
            tps = ep_pool.tile([128, G], f32, tag="ep")
            nc.tensor.transpose(out=tps[:], in_=ar_sb[:, :],
                                identity=ident_sb[:G, :G])
            resT = fin_pool.tile([128, G], f32, tag="resT")
            nc.vector.tensor_copy(out=resT[:], in_=tps[:])
            ops = ep_pool.tile([G, F], f32, tag="ep")
            nc.tensor.matmul(ops[:], lhsT=resT[:], rhs=w012[:], start=True,
                             stop=True)
            icnt = fin_pool.tile([G, 1], f32, tag="icnt")
            nc.sync.dma_start(out=icnt[:], in_=inv_cnt[:])
            fin = fin_pool.tile([G, F], f32, tag="finout")
            nc.vector.tensor_scalar_mul(fin[:], ops[:], icnt[:])
            nc.sync.dma_start(out=out_ext[:], in_=fin[:])

    nc.compile()
    return nc


def make_in_maps(cfg, aux):
    in_maps = []
    for c in range(cfg.n_cores):
        in_maps.append({
            "gidx16": np.ascontiguousarray(aux["gidx16"][c]),
            "a_sm": np.ascontiguousarray(aux["a_sm"][c]),
            "b_sm": np.ascontiguousarray(aux["b_sm"][c]),
            "oh_hbm": np.ascontiguousarray(aux["oh_hbm"][c]),
            "msg0_hbm": np.ascontiguousarray(aux["msg0_hbm"][c]),
            "selfw": np.ascontiguousarray(aux["selfw"][c]),
            "batchloc": np.ascontiguousarray(aux["batchloc"][c]),
            "Xpb": np.ascontiguousarray(aux["Xpb"][c]),
            "inv_cnt": aux["inv_cnt"],
            "W0T": aux["W0T"], "W1T": aux["W1T"], "W2": aux["W2"],
        })
        if aux["spills"] > 0:
            in_maps[-1]["vmaps"] = np.ascontiguousarray(aux["vmaps"][c])
    return in_maps


_PROGRAM_CACHE = {}


def kernel(**inputs):
    from concourse.bass_utils import run_bass_kernel_spmd

    cfg = FULL_CFG
    x = np.asarray(inputs["x"], dtype=np.float32)
    edge_index = np.asarray(inputs["edge_index"])
    edge_attr = np.asarray(inputs["edge_attr"], dtype=np.float32)
    batch = np.asarray(inputs["batch"])
    Ws = np.asarray(inputs["Ws"], dtype=np.float32)
    bs = np.asarray(inputs["bs"], dtype=np.float32)
    assert not np.any(bs), "nonzero biases not supported by this kernel build"

    aux = host_prep(cfg, x, edge_index, edge_attr, batch, Ws, bs)
    use_virtual = aux["spills"] > 0
    key = ("full", cfg.slots_total, use_virtual)
    if key not in _PROGRAM_CACHE:
        _PROGRAM_CACHE[key] = build_program(cfg, use_virtual=use_virtual)
    nc = _PROGRAM_CACHE[key]
    in_maps = make_in_maps(cfg, aux)
    res = run_bass_kernel_spmd(nc, in_maps, core_ids=list(range(cfg.n_cores)))
    return np.asarray(res.results[0]["out"], dtype=np.float32)


# revision 15
# speedup vs baseline: 1.9197x; 1.0708x over previous
"""Trainium2 Bass kernel for CellGraphSignatureGNN (GCN message passing).

Math: the network is affine per layer: x_{l+1} = A @ x_l @ W_l + 1 b_l^T,
with A = D^-1/2 (Adj + 2I) D^-1/2 (weighted adjacency + improved self loops),
followed by a per-graph mean pool P (and bs == 0 in this problem).  Since A
acts on nodes and W on features:

    out = P A^3 X (W0 W1 W2) / counts

We evaluate left-to-right: u1^T = A^T P^T, u2^T = A^T u1^T, u3^T = A^T u2^T
(64-wide node vectors), then one dense matmul (u3 X) sharded over nodes, a
tiny AllReduce, and the 128x128 weight chain on-device.

Distribution: nodes are re-labeled and bin-packed into 128-node blocks (100
blocks/core x 8 cores) balanced by scatter-degree so every core runs an
IDENTICAL (SPMD) program; per-core data (gather indices, one-hot offsets,
norms) differ.  Edge (r, c) contributes norm_e * u[c] into u_new[r]:
  - gather u[c] rows (256B fp32) from the window-replicated u in HBM via
    dma_gather (int16 indices), round-robined over all 4 SWDGE queues so all
    8 GpSimd Q7 cores generate DMA descriptors in parallel,
  - scale by norm_e on DVE (bf16 out), build a destination one-hot on DVE,
  - scatter = one-hot matmul accumulated in PSUM per 128-dest block,
  - the per-layer u re-replication is FOUR quarter AllGathers (Shared outputs)
    that pipeline with compute: quarter q's AG is issued as soon as its 25
    blocks are final, and the next pass's window-w work only waits for AG w.
Self-loops are applied as local elementwise ops; rare (block,window) segment
overflow edges go through a "virtual block" + per-quarter indirect
scatter-add with CCE.
"""

import numpy as np
import ml_dtypes

BF16 = ml_dtypes.bfloat16

G = 64        # graphs
F = 128       # feature width
LAYERS = 3
PAD_SENT = 30000.0  # destoff/bg sentinel for padded slots (one-hot -> 0)


# --------------------------------------------------------------------------
# configuration
# --------------------------------------------------------------------------
class Cfg:
    def __init__(self, n_nodes, n_edges, n_cores=8, nblk=100, group_sizes=None,
                 seg_chunks=2, n_win=4, vchunks=2, vcap=128, csub=8):
        self.n_nodes = n_nodes
        self.n_edges = n_edges
        self.n_cores = n_cores
        self.nblk = nblk                      # real blocks per core
        self.group_sizes = group_sizes or [13, 12] * 4
        assert sum(self.group_sizes) == nblk
        self.seg_chunks = seg_chunks          # 128-slot chunks per (block, window)
        self.n_win = n_win
        self.vchunks = vchunks                # chunks per window for the virtual blk
        self.vcap = vcap                      # distinct spill destinations
        self.csub = csub                      # chunks per dma_gather call
        self.core_rows = nblk * 128           # permuted rows per core
        self.pn = n_cores * self.core_rows    # total permuted rows
        assert self.core_rows % n_win == 0
        self.qrows = self.core_rows // n_win  # rows per (core, quarter)
        self.win = n_cores * self.qrows       # rows per assembled window
        assert self.win <= 32768
        assert nblk % n_win == 0
        self.blocks_per_q = nblk // n_win
        # groups must tile quarters exactly: groups [2q], [2q+1] cover quarter q
        assert len(self.group_sizes) == 2 * n_win
        for q in range(n_win):
            assert (self.group_sizes[2 * q] + self.group_sizes[2 * q + 1]
                    == self.blocks_per_q)
        # slot layout: for g in groups + [vgroup]: for w in windows: contig slots
        self.gw_chunks = [gs * seg_chunks for gs in self.group_sizes] + [vchunks]
        self.n_groups = len(self.gw_chunks)   # includes virtual group
        self.slots_total = sum(self.gw_chunks) * 128 * n_win
        self.gw_slot_off = {}
        off = 0
        for g, c in enumerate(self.gw_chunks):
            for w in range(n_win):
                self.gw_slot_off[(g, w)] = off
                off += c * 128
        assert off == self.slots_total


FULL_CFG = Cfg(100000, 640000)


# --------------------------------------------------------------------------
# host-side graph preprocessing (indices, norms, schedules)
# --------------------------------------------------------------------------
def host_prep(cfg, x, edge_index, edge_attr, batch, Ws, bs):
    N, E = cfg.n_nodes, cfg.n_edges
    row = np.asarray(edge_index[0], dtype=np.int64)
    col = np.asarray(edge_index[1], dtype=np.int64)
    w = np.asarray(edge_attr, dtype=np.float32).reshape(-1)
    batch = np.asarray(batch, dtype=np.int64)

    deg = np.zeros(N, dtype=np.float64)
    np.add.at(deg, col, w.astype(np.float64))
    deg += 2.0
    dinv = (1.0 / np.sqrt(deg)).astype(np.float32)
    norm = dinv[row] * w * dinv[col]
    selfnorm = 2.0 * dinv * dinv
    cnt = np.bincount(batch, minlength=G).astype(np.float32)

    # ---- bin-pack nodes into blocks by scatter degree (edges with row == n)
    sdeg = np.bincount(row, minlength=N)
    nbins = cfg.n_cores * cfg.nblk
    order = np.argsort(-sdeg, kind="stable")
    binsum = np.zeros(nbins, dtype=np.int64)
    binfill = np.zeros(nbins, dtype=np.int32)
    import heapq
    heap = [(0, b) for b in range(nbins)]
    heapq.heapify(heap)
    node_bin = np.empty(N, dtype=np.int32)
    node_pos = np.empty(N, dtype=np.int32)
    for n in order:
        while True:
            s, b = heapq.heappop(heap)
            if binfill[b] < 128:
                break
        node_bin[n] = b
        node_pos[n] = binfill[b]
        binfill[b] += 1
        binsum[b] += sdeg[n]
        if binfill[b] < 128:
            heapq.heappush(heap, (int(binsum[b]), b))
    # snake-assign bins to cores by load
    border = np.argsort(-binsum, kind="stable")
    bin_core = np.empty(nbins, dtype=np.int32)
    bin_blk = np.empty(nbins, dtype=np.int32)
    percore = [[] for _ in range(cfg.n_cores)]
    for i, b in enumerate(border):
        r = i // cfg.n_cores
        k = i % cfg.n_cores
        c = k if (r % 2 == 0) else cfg.n_cores - 1 - k
        bin_core[b] = c
        bin_blk[b] = len(percore[c])
        percore[c].append(b)
    assert all(len(p) == cfg.nblk for p in percore)

    # global permuted row; window = contiguous slice of the AllGather output
    local_row = bin_blk[node_bin].astype(np.int64) * 128 + node_pos
    node_core = bin_core[node_bin].astype(np.int64)
    perm = node_core * cfg.core_rows + local_row
    node_q = perm // cfg.win
    node_widx = perm % cfg.win

    # ---- per-core edge schedules
    e_core = bin_core[node_bin[row]]
    e_blk = bin_blk[node_bin[row]]
    e_doff = node_pos[row]                 # dest offset within block
    e_win = node_q[col].astype(np.int32)   # source window (quarter)
    e_gidx = node_widx[col].astype(np.int32)  # gather idx within window

    SEG = cfg.seg_chunks * 128
    S = cfg.slots_total
    n_cores = cfg.n_cores

    gidx = np.zeros((n_cores, S), dtype=np.int32)
    doff = np.full((n_cores, S), PAD_SENT, dtype=np.float32)
    nrm = np.zeros((n_cores, S), dtype=np.float32)
    bg = np.full((n_cores, S), PAD_SENT, dtype=np.float32)

    # block -> (group, block-in-group) mapping
    blk_group = []
    blk_ing = []
    for g, gs in enumerate(cfg.group_sizes):
        for j in range(gs):
            blk_group.append(g)
            blk_ing.append(j)
    blk_group = np.array(blk_group)
    blk_ing = np.array(blk_ing)

    # virtual spill map: local shard row per spill slot (1<<30 = inactive)
    vmaps = np.full((n_cores, 128, 1), 1 << 30, dtype=np.int32)
    spill_warn = 0

    for c in range(n_cores):
        em = e_core == c
        eb = e_blk[em]
        ew = e_win[em]
        eg = e_gidx[em]
        ed = e_doff[em]
        en = norm[em]
        ebg = batch[col[em]].astype(np.float32)
        # order edges by (block, window)
        key = eb * cfg.n_win + ew
        o = np.argsort(key, kind="stable")
        eb, ew, eg, ed, en, ebg = eb[o], ew[o], eg[o], ed[o], en[o], ebg[o]
        spill_list = []
        kk = eb * cfg.n_win + ew
        bounds = np.searchsorted(kk, np.arange(cfg.nblk * cfg.n_win + 1))
        vused = {}
        for b in range(cfg.nblk):
            g = blk_group[b]
            j = blk_ing[b]
            for wi in range(cfg.n_win):
                lo, hi = bounds[b * cfg.n_win + wi], bounds[b * cfg.n_win + wi + 1]
                nseg = hi - lo
                take = min(nseg, SEG)
                base = (cfg.gw_slot_off[(g, wi)] + j * SEG)
                sl = slice(base, base + take)
                gidx[c, sl] = eg[lo:lo + take]
                doff[c, sl] = ed[lo:lo + take]
                nrm[c, sl] = en[lo:lo + take]
                bg[c, sl] = ebg[lo:lo + take]
                for t in range(lo + take, hi):
                    spill_list.append((b, ew[t], eg[t], ed[t], en[t], ebg[t]))
        # spills into virtual group
        vg = cfg.n_groups - 1
        vfill = np.zeros(cfg.n_win, dtype=np.int32)
        for (b, wi, gg, dd, nn, bb) in spill_list:
            key2 = (b, dd)
            if key2 not in vused:
                assert len(vused) < cfg.vcap, "virtual dest capacity exceeded"
                v = len(vused)
                vused[key2] = v
                vmaps[c, v, 0] = b * 128 + dd
            v = vused[key2]
            assert vfill[wi] < cfg.vchunks * 128, "virtual slot capacity exceeded"
            base = cfg.gw_slot_off[(vg, wi)] + vfill[wi]
            gidx[c, base] = gg
            doff[c, base] = v
            nrm[c, base] = nn
            bg[c, base] = bb
            vfill[wi] += 1
        spill_warn += len(spill_list)

    # ---- pack aux arrays
    ncol16 = S // 16
    gidx16 = np.zeros((n_cores, 128, ncol16), dtype=np.int16)
    s_idx = np.arange(S)
    for c in range(n_cores):
        lay = np.zeros((16, ncol16), dtype=np.int16)
        lay[s_idx % 16, s_idx // 16] = (gidx[c] // 2).astype(np.int16)
        gidx16[c] = np.tile(lay, (8, 1))
    ncol128 = S // 128

    def slotmajor(a, dt):
        out = np.zeros((n_cores, 128, ncol128), dtype=dt)
        for c in range(n_cores):
            out[c][s_idx % 128, s_idx // 128] = a[c]
        return out

    nrm_sm = slotmajor(nrm, np.float32)
    doff_sm = slotmajor(doff, BF16)
    bg_sm = slotmajor(bg, BF16)
    # pair-gather: index = source-pair row; a/b select the even/odd half
    odd = (gidx % 2).astype(np.float32)
    a_sm = slotmajor(nrm * (1.0 - odd), BF16)
    b_sm = slotmajor(nrm * odd, BF16)

    # host-prebuilt destination one-hots (all passes) and pass-0 messages
    SC = S // 128
    p_i = s_idx % 128
    c_i = s_idx // 128
    oh_hbm = np.zeros((n_cores, 128, SC, 128), dtype=BF16)
    msg0_hbm = np.zeros((n_cores, 128, SC, G), dtype=BF16)
    for c in range(n_cores):
        dv = doff[c].astype(np.int64)
        m = doff[c] != PAD_SENT
        oh_hbm[c][p_i[m], c_i[m], dv[m]] = 1
        bv = bg[c].astype(np.int64)
        mb = bg[c] != PAD_SENT
        msg0_hbm[c][p_i[mb], c_i[mb], bv[mb]] = nrm[c][mb].astype(BF16)

    # ---- per-core node-level aux
    selfw = np.zeros((n_cores, 128, cfg.nblk), dtype=np.float32)
    batchloc = np.full((n_cores, 128, cfg.nblk), PAD_SENT, dtype=BF16)
    Xp = np.zeros((n_cores, cfg.core_rows, F), dtype=np.float32)
    nodes = np.arange(N)
    pc = bin_core[node_bin]
    pb = bin_blk[node_bin]
    pp = node_pos
    for c in range(n_cores):
        m = pc == c
        selfw[c][pp[m], pb[m]] = selfnorm[nodes[m]]
        batchloc[c][pp[m], pb[m]] = batch[nodes[m]].astype(np.float32)
        Xp[c][pb[m] * 128 + pp[m]] = np.asarray(x, dtype=np.float32)[nodes[m]]

    inv_cnt = (1.0 / np.maximum(cnt, 1.0)).astype(np.float32).reshape(G, 1)
    Ws = np.asarray(Ws, dtype=np.float32)
    bs = np.asarray(bs, dtype=np.float32)

    aux = dict(
        gidx16=gidx16, nrm_sm=nrm_sm, doff_sm=doff_sm, bg_sm=bg_sm,
        a_sm=a_sm, b_sm=b_sm, Xpb=Xp.astype(BF16),
        oh_hbm=oh_hbm, msg0_hbm=msg0_hbm,
        selfw=selfw, batchloc=batchloc, Xp=Xp, vmaps=vmaps,
        inv_cnt=inv_cnt,
        W0T=np.ascontiguousarray(Ws[0].T), W1T=np.ascontiguousarray(Ws[1].T),
        W2=np.ascontiguousarray(Ws[2]), bs=bs,
        spills=spill_warn,
    )
    return aux


def _midbcast(ap, count):
    """Insert a step-0 middle axis: [P, X] -> [P, (0,count), X]."""
    import concourse.bass as bass
    assert len(ap.ap) == 2
    return bass.AP(ap.tensor, ap.offset, [ap.ap[0], [0, count], ap.ap[1]])


def build_program(cfg, use_virtual=True):
    import contextlib
    import concourse.bacc as bacc
    import concourse.bass as bass
    import concourse.mybir as mybir
    import concourse.tile as tile

    f32 = mybir.dt.float32
    bf16 = mybir.dt.bfloat16
    i16 = mybir.dt.int16
    i32 = mybir.dt.int32
    AL = mybir.AluOpType

    S = cfg.slots_total
    SC = S // 128
    NBLK = cfg.nblk
    NW = cfg.n_win
    SEGC = cfg.seg_chunks
    BPQ = cfg.blocks_per_q
    QR = cfg.qrows
    gbase = [0]
    for gs in cfg.group_sizes:
        gbase.append(gbase[-1] + gs)

    nc = bacc.Bacc("TRN2", debug=False, num_devices=cfg.n_cores,
                   num_swdge_queues=4)
    P = nc.declare_dram_parameter

    gidx16 = P("gidx16", [128, S // 16], i16, isOutput=False)
    a_sm = P("a_sm", [128, SC], bf16, isOutput=False)
    b_sm = P("b_sm", [128, SC], bf16, isOutput=False)
    oh_hbm = P("oh_hbm", [128, SC, 128], bf16, isOutput=False)
    msg0_hbm = P("msg0_hbm", [128, SC, G], bf16, isOutput=False)
    selfw = P("selfw", [128, NBLK], f32, isOutput=False)
    batchloc = P("batchloc", [128, NBLK], bf16, isOutput=False)
    Xp = P("Xpb", [cfg.core_rows, F], bf16, isOutput=False)
    if use_virtual:
        vmaps = P("vmaps", [128, 1], i32, isOutput=False)
    inv_cnt = P("inv_cnt", [G, 1], f32, isOutput=False)
    W0T = P("W0T", [F, F], f32, isOutput=False)
    W1T = P("W1T", [F, F], f32, isOutput=False)
    W2 = P("W2", [F, F], f32, isOutput=False)
    out_ext = P("out", [G, F], f32, isOutput=True)

    # node-sharded output (AG input / final u3) and replicated u (AG out)
    shard = nc.dram_tensor("shard", [cfg.core_rows, G], bf16)
    ufull = [nc.dram_tensor(f"ufull{p}", [cfg.pn, G], bf16) for p in range(2)]
    arin = nc.dram_tensor("arin", [G, F], f32)
    arout = nc.dram_tensor("arout", [G, F], f32)

    iota64_c = nc.inline_tensor(
        np.tile(np.arange(G, dtype=np.float32).astype(BF16), (128, 1)), "iota64")
    ident_c = nc.inline_tensor(np.eye(128, dtype=np.float32), "ident")

    qn = [0]

    def next_q():
        qn[0] = (qn[0] + 1) % 4
        return qn[0]

    with tile.TileContext(nc) as tc:
        with contextlib.ExitStack() as ctx:
            perm_pool = ctx.enter_context(tc.tile_pool(name="perm", bufs=1))
            acc = perm_pool.tile([128, NBLK, G], bf16, tag="acc")
            selfw_sb = perm_pool.tile([128, NBLK], f32, tag="selfw")
            blc_sb = perm_pool.tile([128, NBLK], bf16, tag="blc")
            io64 = perm_pool.tile([128, G], bf16, tag="io64")
            ident_sb = perm_pool.tile([128, 128], f32, tag="ident")
            w_sb = perm_pool.tile([128, 3 * F], f32, tag="wsb")

            nc.sync.dma_start(out=selfw_sb[:], in_=selfw[:])
            nc.sync.dma_start(out=blc_sb[:], in_=batchloc[:])
            nc.sync.dma_start(out=io64[:], in_=iota64_c[:])
            nc.sync.dma_start(out=ident_sb[:], in_=ident_c[:])
            nc.sync.dma_start(out=w_sb[:, 0:F], in_=W0T[:])
            nc.sync.dma_start(out=w_sb[:, F:2 * F], in_=W1T[:])
            nc.sync.dma_start(out=w_sb[:, 2 * F:3 * F], in_=W2[:])

            raw_pool = ctx.enter_context(tc.tile_pool(name="raw", bufs=6))
            msg_pool = ctx.enter_context(tc.tile_pool(name="msg", bufs=6))
            oh_pool = ctx.enter_context(tc.tile_pool(name="oh", bufs=6))
            up_pool = ctx.enter_context(tc.tile_pool(name="up", bufs=2))
            ps_pool = ctx.enter_context(tc.tile_pool(name="ps", bufs=2, space="PSUM"))
            ep_pool = ctx.enter_context(tc.tile_pool(name="ep", bufs=2, space="PSUM"))
            fin_pool = ctx.enter_context(tc.tile_pool(name="fin", bufs=2))
            xp_pool = ctx.enter_context(tc.tile_pool(name="xp", bufs=2))
            if use_virtual:
                vps_pool = ctx.enter_context(
                    tc.tile_pool(name="vps", bufs=2, space="PSUM"))
                vmap_sb = perm_pool.tile([128, 1], i32, tag="vmap")
                nc.sync.dma_start(out=vmap_sb[:], in_=vmaps[:])

            # static per-slot aux data, resident in SBUF for all passes
            gidx_sb = perm_pool.tile([128, S // 16], i16, tag="gidx")
            a_sb = perm_pool.tile([128, SC], bf16, tag="asb")
            b_sb = perm_pool.tile([128, SC], bf16, tag="bsb")
            nc.sync.dma_start(out=gidx_sb[:], in_=gidx16[:])
            nc.sync.dma_start(out=a_sb[:], in_=a_sm[:])
            nc.sync.dma_start(out=b_sb[:], in_=b_sm[:])

            # epilogue weight chain early (overlaps pass 0):
            # W12 = W1 @ W2 ; W012 = W0 @ W12
            wps = ep_pool.tile([128, F], f32, tag="ep")
            w12 = fin_pool.tile([128, F], f32, tag="w12")
            nc.tensor.matmul(wps[:], lhsT=w_sb[:, F:2 * F],
                             rhs=w_sb[:, 2 * F:3 * F], start=True, stop=True)
            nc.vector.tensor_copy(out=w12[:], in_=wps[:])
            wps2 = ep_pool.tile([128, F], f32, tag="ep")
            w012 = perm_pool.tile([128, F], f32, tag="w012")
            nc.tensor.matmul(wps2[:], lhsT=w_sb[:, 0:F], rhs=w12[:],
                             start=True, stop=True)
            nc.vector.tensor_copy(out=w012[:], in_=wps2[:])

            # [core_rows, G] viewed as [128p, NBLK, G]
            shard_pbf = shard[:].rearrange("(b p) f -> p b f", p=128)
            max_gs = max(cfg.group_sizes)

            def build_msgs(pk, g, w, C, soff):
                """Returns (msgt, oht) for the (g, w) slot region."""
                coff = soff // 128
                oht = oh_pool.tile([128, C, 128], bf16, tag="oh")
                nc.scalar.dma_start(out=oht[:, :, :],
                                    in_=oh_hbm[:, coff:coff + C, :])
                if pk == 0:
                    msgt = msg_pool.tile([128, C, G], bf16, tag="msg")
                    nc.sync.dma_start(out=msgt[:, :, :],
                                      in_=msg0_hbm[:, coff:coff + C, :])
                else:
                    src = ufull[(pk + 1) % 2][
                        w * cfg.win:(w + 1) * cfg.win, :].rearrange(
                        "(p two) f -> p (two f)", two=2)
                    rawt = raw_pool.tile([128, C, 2 * G], bf16, tag="raw")
                    for sub in range(0, C, cfg.csub):
                        cs = min(cfg.csub, C - sub)
                        nc.gpsimd.dma_gather(
                            rawt[:, sub:sub + cs, :],
                            src,
                            gidx_sb[:, soff // 16 + sub * 8:
                                    soff // 16 + (sub + cs) * 8],
                            cs * 128, cs * 128, 2 * G,
                            single_packet=False,
                            queue_num=next_q())
                    msgt = msg_pool.tile([128, C, G], bf16, tag="msg")
                    tmpt = msg_pool.tile([128, C, G], bf16, tag="msgtmp")
                    nc.vector.tensor_tensor(
                        out=msgt[:, :, :], in0=rawt[:, :, 0:G],
                        in1=a_sb[:, coff:coff + C].to_broadcast([128, C, G]),
                        op=AL.mult)
                    nc.vector.tensor_tensor(
                        out=tmpt[:, :, :], in0=rawt[:, :, G:2 * G],
                        in1=b_sb[:, coff:coff + C].to_broadcast([128, C, G]),
                        op=AL.mult)
                    nc.vector.tensor_tensor(
                        out=msgt[:, :, :], in0=msgt[:, :, :],
                        in1=tmpt[:, :, :], op=AL.add)
                return msgt, oht

            def do_pass(pk):
                # virtual (spill) group first so vsb is ready per quarter
                if use_virtual:
                    vg = cfg.n_groups - 1
                    VC = cfg.vchunks
                    vsb = fin_pool.tile([128, G], f32, tag="vsb")
                    for w in range(NW):
                        soff = cfg.gw_slot_off[(vg, w)]
                        msgt, oht = build_msgs(pk, vg, w, VC, soff)
                        vps = vps_pool.tile([128, G], f32, tag="vps")
                        for ci in range(VC):
                            nc.tensor.matmul(
                                vps[:, :], lhsT=oht[:, ci, :],
                                rhs=msgt[:, ci, :],
                                start=(ci == 0), stop=(ci == VC - 1))
                        if w == 0:
                            nc.vector.tensor_copy(out=vsb[:], in_=vps[:])
                        else:
                            nc.vector.tensor_tensor(
                                out=vsb[:], in0=vsb[:], in1=vps[:], op=AL.add)

                for qo in range(NW):
                    for g in (2 * qo, 2 * qo + 1):
                        gs = cfg.group_sizes[g]
                        C = cfg.gw_chunks[g]
                        seg = slice(gbase[g], gbase[g] + gs)
                        # self-loop init of acc for this group
                        upt = up_pool.tile([128, gs, G], bf16, tag="up")
                        if pk == 0:
                            nc.vector.tensor_tensor(
                                out=upt[:, :, :],
                                in0=blc_sb[:, seg].to_broadcast([128, gs, G]),
                                in1=_midbcast(io64[:], gs),
                                op=AL.is_equal)
                        else:
                            nc.sync.dma_start(
                                out=upt[:, :, :],
                                in_=shard_pbf[:, gbase[g]:gbase[g] + gs, :])
                        nc.vector.tensor_tensor(
                            out=acc[:, seg, :], in0=upt[:, :, :],
                            in1=selfw_sb[:, seg].to_broadcast([128, gs, G]),
                            op=AL.mult)
                        # all 4 windows' msgs, then one contiguous
                        # accumulation chain per PSUM region
                        mos = [build_msgs(pk, g, w, C, cfg.gw_slot_off[(g, w)])
                               for w in range(NW)]
                        ps = ps_pool.tile([128, max_gs * G], f32, tag="ps")
                        for j in range(gs):
                            for w in range(NW):
                                msgt, oht = mos[w]
                                for s in range(SEGC):
                                    ci = j * SEGC + s
                                    nc.tensor.matmul(
                                        ps[:, j * G:(j + 1) * G],
                                        lhsT=oht[:, ci, :], rhs=msgt[:, ci, :],
                                        start=(w == 0 and s == 0),
                                        stop=(w == NW - 1 and s == SEGC - 1))
                        nc.vector.tensor_tensor(
                            out=acc[:, seg, :], in0=acc[:, seg, :],
                            in1=ps[:, :gs * G].rearrange(
                                "p (b f) -> p b f", f=G),
                            op=AL.add)
                    # quarter qo finalized: write its shard rows (not
                    # needed in the last pass unless spills must be applied)
                    if pk < LAYERS - 1 or use_virtual:
                        nc.sync.dma_start(
                            out=shard_pbf[:, qo * BPQ:(qo + 1) * BPQ, :],
                            in_=acc[:, qo * BPQ:(qo + 1) * BPQ, :])
                if use_virtual:
                    nc.gpsimd.indirect_dma_start(
                        out=shard[:, :],
                        out_offset=bass.IndirectOffsetOnAxis(
                            ap=vmap_sb[:, 0:1], axis=0),
                        in_=vsb[:, :], in_offset=None,
                        bounds_check=cfg.core_rows - 1, oob_is_err=False,
                        compute_op=AL.add)
                if pk < LAYERS - 1:
                    nc.gpsimd.collective_compute(
                        "AllGather", AL.bypass,
                        replica_groups=[list(range(cfg.n_cores))],
                        ins=[shard[:]], outs=[ufull[pk % 2][:]])

            for pk in range(LAYERS):
                do_pass(pk)

            # final dense matmul: out_part[g, f] = sum_n u3[n, g] * Xp[n, f]
            fps = ep_pool.tile([G, F], f32, tag="ep")
            xp_pbf = Xp[:].rearrange("(b p) f -> p b f", p=128)
            if use_virtual:
                u3 = perm_pool.tile([128, NBLK, G], bf16, tag="u3")
                nc.sync.dma_start(out=u3[:, :, :], in_=shard_pbf)
            else:
                u3 = acc
            for qo in range(NW):
                xpt = xp_pool.tile([128, BPQ, F], bf16, tag="xp")
                nc.sync.dma_start(
                    out=xpt[:, :, :],
                    in_=xp_pbf[:, qo * BPQ:(qo + 1) * BPQ, :])
                for b in range(BPQ):
                    gb = qo * BPQ + b
                    nc.tensor.matmul(fps[:], lhsT=u3[:, qo * BPQ + b, :],
                                     rhs=xpt[:, b, :],
                                     start=(gb == 0), stop=(gb == NBLK - 1))
            outp = fin_pool.tile([G, F], f32, tag="outp")
            nc.vector.tensor_copy(out=outp[:], in_=fps[:])
            nc.sync.dma_start(out=arin[:], in_=outp[:])
            nc.gpsimd.collective_compute(
                "AllReduce", AL.add,
                replica_groups=[list(range(cfg.n_cores))],
                ins=[arin[:]], outs=[arout[:]])
            ar_sb = fin_pool.tile([G, F], f32, tag="arsb")
            nc.sync.dma_start(out=ar_sb[:], in_=arout[:])

            # epilogue: res^T ; out = (res^T W012) / counts
            tps = ep_pool.tile([128, G], f32, tag="ep")
            nc.tensor.transpose(out=tps[:], in_=ar_sb[:, :],
                                identity=ident_sb[:G, :G])
            resT = fin_pool.tile([128, G], f32, tag="resT")
            nc.vector.tensor_copy(out=resT[:], in_=tps[:])
            ops = ep_pool.tile([G, F], f32, tag="ep")
            nc.tensor.matmul(ops[:], lhsT=resT[:], rhs=w012[:], start=True,
                             stop=True)
            icnt = fin_pool.tile([G, 1], f32, tag="icnt")
            nc.sync.dma_start(out=icnt[:], in_=inv_cnt[:])
            fin = fin_pool.tile([G, F], f32, tag="finout")
            nc.vector.tensor_scalar_mul(fin[:], ops[:], icnt[:])
            nc.sync.dma_start(out=out_ext[:], in_=fin[:])

    nc.compile()
    return nc


def make_in_maps(cfg, aux):
    in_maps = []
    for c in range(cfg.n_cores):
        in_maps.append({
            "gidx16": np.ascontiguousarray(aux["gidx16"][c]),
            "a_sm": np.ascontiguousarray(aux["a_sm"][c]),
            "b_sm": np.ascontiguousarray(aux["b_sm"][c]),
            "oh_hbm": np.ascontiguousarray(aux["oh_hbm"][c]),
            "msg0_hbm": np.ascontiguousarray(aux["msg0_hbm"][c]),
            "selfw": np.ascontiguousarray(aux["selfw"][c]),
            "batchloc": np.ascontiguousarray(aux["batchloc"][c]),
            "Xpb": np.ascontiguousarray(aux["Xpb"][c]),
            "inv_cnt": aux["inv_cnt"],
            "W0T": aux["W0T"], "W1T": aux["W1T"], "W2": aux["W2"],
        })
        if aux["spills"] > 0:
            in_maps[-1]["vmaps"] = np.ascontiguousarray(aux["vmaps"][c])
    return in_maps


_PROGRAM_CACHE = {}


def kernel(**inputs):
    from concourse.bass_utils import run_bass_kernel_spmd

    cfg = FULL_CFG
    x = np.asarray(inputs["x"], dtype=np.float32)
    edge_index = np.asarray(inputs["edge_index"])
    edge_attr = np.asarray(inputs["edge_attr"], dtype=np.float32)
    batch = np.asarray(inputs["batch"])
    Ws = np.asarray(inputs["Ws"], dtype=np.float32)
    bs = np.asarray(inputs["bs"], dtype=np.float32)
    assert not np.any(bs), "nonzero biases not supported by this kernel build"

    aux = host_prep(cfg, x, edge_index, edge_attr, batch, Ws, bs)
    use_virtual = aux["spills"] > 0
    key = ("full", cfg.slots_total, use_virtual)
    if key not in _PROGRAM_CACHE:
        _PROGRAM_CACHE[key] = build_program(cfg, use_virtual=use_virtual)
    nc = _PROGRAM_CACHE[key]
    in_maps = make_in_maps(cfg, aux)
    res = run_bass_kernel_spmd(nc, in_maps, core_ids=list(range(cfg.n_cores)))
    return np.asarray(res.results[0]["out"], dtype=np.float32)
